# revision 8
# baseline (speedup 1.0000x reference)
"""Trainium2 Bass kernel for nn_Actor (RSNorm -> Linear -> 4x residual LN-MLP
blocks -> post-LN -> clipped mu/std heads), data-parallel over batch on 8
NeuronCores.

Strategy:
- Shard batch B=16384 into 8x2048 rows; weights replicated per core.
- RSNorm (Welford scan over batch) == population mean/var over batch; computed
  via per-shard bn_stats merged across cores with a tiny (4KB) AllReduce.
- All norms are folded into the adjacent matmuls: per-feature affine goes into
  the weight matrix, per-row (mean, std) corrections enter the PSUM
  accumulation as rank-2 matmuls, and the per-row 1/std scale commutes with
  ReLU so it is applied once on the residual update.
- Activations live feature-major ([feat partitions x row free]) so the whole
  residual trunk needs zero transposes; the heads flip back to row-major by
  using the activation tiles as the stationary matmul operand.
- Matmul compute in bf16 (fp32 PSUM accumulate); residual stream stored bf16.
"""

import sys

if "/opt/trn_rl_repo" not in sys.path:
    sys.path.insert(0, "/opt/trn_rl_repo")

import numpy as np

import concourse.bass as bass
import concourse.bacc as bacc
import concourse.mybir as mybir
from concourse import tile
from concourse.bass_utils import run_bass_kernel_spmd

# bass_utils imports antenv.axon_hooks when tracing is requested via
# BASS_TRACE; provide a no-op fallback module when the image lacks it.
try:
    import antenv.axon_hooks  # noqa: F401
except Exception:
    try:
        import types as _types
        import antenv as _antenv

        _m = _types.ModuleType("antenv.axon_hooks")
        _m.get_axon_ntff_profile_hook = lambda: None
        _m.set_axon_ntff_profile_hook = lambda h: None
        _antenv.axon_hooks = _m
        sys.modules["antenv.axon_hooks"] = _m
    except Exception:
        pass

F32 = mybir.dt.float32
BF16 = mybir.dt.bfloat16
AF = mybir.ActivationFunctionType
ALU = mybir.AluOpType

B, DIN, H, A, L = 16384, 512, 1024, 128, 4
NCORES = 8
R = B // NCORES          # 2048 rows per core
CH = 4                   # chunks per core
CW = R // CH             # 512 rows per chunk
KD = DIN // 128          # 4 k-tiles of the input dim
KH = H // 128            # 8 k-tiles of the hidden dim
EPS_RS = 1e-5
EPS_LN = 1e-5

_COMPILED = None


def _build():
    nc = bacc.Bacc("TRN2", target_bir_lowering=False, debug=False,
                   num_devices=NCORES)

    stated = nc.dram_tensor("state", [R, DIN], F32, kind="ExternalInput")
    W_ind = nc.dram_tensor("W_in", [DIN, H], F32, kind="ExternalInput")
    b_ind = nc.dram_tensor("b_in", [1, H], F32, kind="ExternalInput")
    ln_gd = nc.dram_tensor("ln_g", [L, H], F32, kind="ExternalInput")
    ln_bd = nc.dram_tensor("ln_b", [L, H], F32, kind="ExternalInput")
    W1d = nc.dram_tensor("W1", [L, H, H], F32, kind="ExternalInput")
    b1d = nc.dram_tensor("b1", [L, H], F32, kind="ExternalInput")
    W2d = nc.dram_tensor("W2", [L, H, H], F32, kind="ExternalInput")
    b2d = nc.dram_tensor("b2", [L, H], F32, kind="ExternalInput")
    post_gd = nc.dram_tensor("post_g", [1, H], F32, kind="ExternalInput")
    post_bd = nc.dram_tensor("post_b", [1, H], F32, kind="ExternalInput")
    Wmud = nc.dram_tensor("Wmu", [H, A], F32, kind="ExternalInput")
    bmud = nc.dram_tensor("bmu", [1, A], F32, kind="ExternalInput")
    Wstdd = nc.dram_tensor("Wstd", [H, A], F32, kind="ExternalInput")
    bstdd = nc.dram_tensor("bstd", [1, A], F32, kind="ExternalInput")
    outd = nc.dram_tensor("out", [R, 2 * A], F32, kind="ExternalOutput")

    identd = nc.inline_tensor(np.eye(128, dtype=np.float32), name="ident")

    # register 1e-5 as a const AP so activation(bias=eps) resolves
    eps_t = nc.alloc_sbuf_tensor("const-eps", [128, 1], F32)
    nc.gpsimd.memset(eps_t.ap(), EPS_LN)
    nc.const_aps.aps[(F32, EPS_LN)] = eps_t.ap()
    nc.all_engine_barrier()

    with tile.TileContext(nc) as tc:
        with (
            tc.tile_pool(name="const", bufs=1) as cp,
            tc.tile_pool(name="xp", bufs=1) as xp,
            tc.tile_pool(name="wp", bufs=2) as wp,
            tc.tile_pool(name="ap", bufs=3) as ap,
            tc.tile_pool(name="rp", bufs=2) as rp,
            tc.tile_pool(name="sp", bufs=4) as sp,
            tc.tile_pool(name="psA", bufs=3, space="PSUM") as psA,
            tc.tile_pool(name="psS", bufs=1, space="PSUM") as psS,
            tc.tile_pool(name="psB", bufs=2, space="PSUM") as psB,
            tc.tile_pool(name="psT", bufs=1, space="PSUM") as psT,
            tc.tile_pool(name="dp", bufs=1, space="DRAM") as dp,
        ):
            # ---------------- constants ----------------
            ones128 = cp.tile([128, 128], BF16)
            nc.vector.memset(ones128[:], 1.0)
            onesrow = cp.tile([1, CW], BF16)
            nc.vector.memset(onesrow[:], 1.0)
            ident = cp.tile([128, 128], F32)
            nc.sync.dma_start(ident[:], identd[:])

            # ln_g/ln_b interleaved column tiles: [128, L*KH*2]
            glb_bf = cp.tile([128, L * KH * 2], BF16)
            glb_f = cp.tile([128, L * KH * 2], F32)
            for src, off in ((ln_gd, 0), (ln_bd, 1)):
                view = src[:].rearrange("l (k p) -> p (l k)", p=128)
                dst_bf = glb_bf[:].rearrange("p (lk two) -> p lk two", two=2)
                dst_f = glb_f[:].rearrange("p (lk two) -> p lk two", two=2)
                nc.gpsimd.dma_start(dst_bf[:, :, off], view)
                nc.gpsimd.dma_start(dst_f[:, :, off], view)
            # post_g/post_b column tiles: [128, KH*2]
            pglb_bf = cp.tile([128, KH * 2], BF16)
            for src, off in ((post_gd, 0), (post_bd, 1)):
                view = src[:].rearrange("o (k p) -> p (o k)", p=128)
                dst = pglb_bf[:].rearrange("p (k two) -> p k two", two=2)
                nc.gpsimd.dma_start(dst[:, :, off], view)

            # bias rows
            b_in_row = cp.tile([1, H], F32)
            nc.sync.dma_start(b_in_row[:], b_ind[:])
            bhead = cp.tile([1, 2 * A], F32)
            nc.sync.dma_start(bhead[:, 0:A], bmud[:])
            nc.sync.dma_start(bhead[:, A:2 * A], bstdd[:])

            # ---------------- stage A: state load + transpose ----------------
            xt = [[xp.tile([128, CW], BF16, tag=f"xt_{k}_{c}", name=f"xt_{k}_{c}")
                   for c in range(CH)] for k in range(KD)]
            for c in range(CH):
                for j in range(4):
                    srow = ap.tile([128, DIN], BF16, tag="srow", bufs=4)
                    nc.gpsimd.dma_start(
                        srow[:], stated[(c * 4 + j) * 128:(c * 4 + j + 1) * 128, :])
                    for k in range(KD):
                        nc.sync.dma_start_transpose(
                            xt[k][c][:, j * 128:(j + 1) * 128],
                            srow[:, k * 128:(k + 1) * 128])

            # ---------------- rsnorm stats + allreduce ----------------
            allin = sp.tile([128, KD * 2], F32, tag="allin", bufs=1)
            for k in range(KD):
                bnbuf = sp.tile([128, CH * 6], F32, tag=f"bn_{k}", bufs=1)
                for c in range(CH):
                    nc.vector.bn_stats(bnbuf[:, c * 6:(c + 1) * 6], xt[k][c][:])
                aggr = sp.tile([128, 2], F32, tag=f"aggr_{k}", bufs=1)
                nc.vector.bn_aggr(
                    aggr[:], bnbuf[:].rearrange("p (c s) -> p c s", s=6))
                # sum = mean * R ; sumsq = (var + mean^2) * R
                nc.scalar.activation(allin[:, 2 * k:2 * k + 1], aggr[:, 0:1],
                                     AF.Copy, scale=float(R))
                t1 = sp.tile([128, 1], F32, tag="t1")
                nc.vector.tensor_tensor(t1[:], aggr[:, 0:1], aggr[:, 0:1],
                                        op=ALU.mult)
                nc.vector.tensor_tensor(t1[:], t1[:], aggr[:, 1:2], op=ALU.add)
                nc.scalar.activation(allin[:, 2 * k + 1:2 * k + 2], t1[:],
                                     AF.Copy, scale=float(R))
            cc_in = dp.tile([128, KD * 2], F32)
            cc_out = dp.tile([128, KD * 2], F32, addr_space="Shared")
            nc.gpsimd.dma_start(cc_in[:], allin[:])
            nc.gpsimd.collective_compute(
                "AllReduce", ALU.add,
                replica_groups=[list(range(NCORES))],
                ins=[cc_in[:].opt()], outs=[cc_out[:].opt()])
            allout = sp.tile([128, KD * 2], F32, tag="allout", bufs=1)
            nc.gpsimd.dma_start(allout[:], cc_out[:])

            # per-feature fold factors for W_in
            a_col = []
            c_col = []
            for k in range(KD):
                muk = sp.tile([128, 1], F32, tag=f"muk_{k}", bufs=1)
                nc.scalar.activation(muk[:], allout[:, 2 * k:2 * k + 1],
                                     AF.Copy, scale=1.0 / B)
                var = sp.tile([128, 1], F32, tag="var1")
                nc.scalar.activation(var[:], allout[:, 2 * k + 1:2 * k + 2],
                                     AF.Copy, scale=1.0 / B)
                msq = sp.tile([128, 1], F32, tag="msq1")
                nc.vector.tensor_tensor(msq[:], muk[:], muk[:], op=ALU.mult)
                nc.vector.tensor_tensor(var[:], var[:], msq[:], op=ALU.subtract)
                nc.vector.tensor_scalar_max(var[:], var[:], 0.001)
                nc.scalar.activation(var[:], var[:], AF.Ln, bias=EPS_RS)
                ak = sp.tile([128, 1], F32, tag=f"ak_{k}", bufs=1)
                nc.scalar.activation(ak[:], var[:], AF.Exp, scale=-0.5)
                mak = sp.tile([128, 1], F32, tag="mak")
                nc.vector.tensor_tensor(mak[:], muk[:], ak[:], op=ALU.mult)
                ck = sp.tile([128, 1], BF16, tag=f"ck_{k}", bufs=1)
                nc.scalar.activation(ck[:], mak[:], AF.Copy, scale=-1.0)
                a_col.append(ak)
                c_col.append(ck)

            # ---------------- W_in fold + x1 ----------------
            w_in = []
            for k in range(KD):
                w = wp.tile([128, H], BF16, tag=f"win_{k}", bufs=1)
                nc.gpsimd.dma_start(w[:], W_ind[k * 128:(k + 1) * 128, :])
                w_in.append(w)
            # dvec = c @ W_in + b_in
            dvec = sp.tile([1, H], BF16, tag="dvec", bufs=1)
            for half in range(2):
                psd = psB.tile([2, 512], F32, tag="small")
                for k in range(KD):
                    nc.tensor.matmul(psd[0:1, :], c_col[k][:],
                                     w_in[k][:, half * 512:(half + 1) * 512],
                                     start=(k == 0), stop=(k == KD - 1))
                nc.vector.tensor_tensor(
                    dvec[:, half * 512:(half + 1) * 512], psd[0:1, :],
                    b_in_row[:, half * 512:(half + 1) * 512], op=ALU.add)
            # W_in <- a * W_in (in place, after dvec matmuls)
            for k in range(KD):
                nc.vector.tensor_scalar(w_in[k][:], w_in[k][:], a_col[k][:],
                                        None, op0=ALU.mult)

            x = [[xp.tile([128, CW], BF16, tag=f"x_{n}_{c}", name=f"x_{n}_{c}")
                  for c in range(CH)] for n in range(KH)]
            for c in range(CH):
                for n in range(KH):
                    ps = psA.tile([128, CW], F32, tag="ps")
                    for k in range(KD):
                        nc.tensor.matmul(ps[:], w_in[k][:, n * 128:(n + 1) * 128],
                                         xt[k][c][:], start=(k == 0), stop=False)
                    nc.tensor.matmul(ps[:], dvec[:, n * 128:(n + 1) * 128],
                                     onesrow[:], start=False, stop=True)
                    nc.scalar.activation(x[n][c][:], ps[:], AF.Copy)

            # ---------------- helper: per-chunk row stats ----------------
            def emit_stats(c, eps):
                """LN row stats over the current x[:, chunk c].

                Returns (q_b [128,CW] f32 = 1/std broadcast, corr [2,CW] bf16
                rows (-mean, std))."""
                pss = psS.tile([128, CW], F32, tag="pss")
                psq = psS.tile([128, CW], F32, tag="psq")
                for k in range(KH):
                    sq = ap.tile([128, CW], BF16, tag="sq")
                    nc.scalar.activation(sq[:], x[k][c][:], AF.Square)
                    nc.tensor.matmul(pss[:], ones128[:], x[k][c][:],
                                     start=(k == 0), stop=(k == KH - 1))
                    nc.tensor.matmul(psq[:], ones128[:], sq[:],
                                     start=(k == 0), stop=(k == KH - 1))
                negm_row = sp.tile([1, CW], BF16, tag="negm")
                nc.scalar.activation(negm_row[:], pss[0:1, :], AF.Copy,
                                     scale=-1.0 / H)
                m_b = ap.tile([128, CW], F32, tag="m_b", bufs=2)
                nc.scalar.activation(m_b[:], pss[:], AF.Copy, scale=1.0 / H)
                nc.vector.tensor_tensor(m_b[:], m_b[:], m_b[:], op=ALU.mult)
                var = ap.tile([128, CW], F32, tag="varb", bufs=2)
                nc.vector.scalar_tensor_tensor(var[:], psq[:], 1.0 / H, m_b[:],
                                               op0=ALU.mult, op1=ALU.subtract)
                nc.scalar.activation(var[:], var[:], AF.Ln, bias=eps)
                q_b = ap.tile([128, CW], F32, tag="qb", bufs=2)
                nc.scalar.activation(q_b[:], var[:], AF.Exp, scale=-0.5)
                s_row = sp.tile([1, CW], BF16, tag="s_row")
                nc.scalar.activation(s_row[:], var[0:1, :], AF.Exp, scale=0.5)
                return q_b, negm_row, s_row

            # ---------------- blocks ----------------
            for l in range(L):
                w1 = []
                w2 = []
                for k in range(KH):
                    w = wp.tile([128, H], BF16, tag=f"w1_{k}")
                    nc.gpsimd.dma_start(w[:], W1d[l, k * 128:(k + 1) * 128, :])
                    w1.append(w)
                for k in range(KH):
                    w = wp.tile([128, H], BF16, tag=f"w2_{k}")
                    nc.gpsimd.dma_start(w[:], W2d[l, k * 128:(k + 1) * 128, :])
                    w2.append(w)
                b1row = sp.tile([1, H], F32, tag="b1row", bufs=2)
                nc.sync.dma_start(b1row[:], b1d[l:l + 1, :])
                b2row = sp.tile([1, H], BF16, tag="b2row", bufs=2)
                nc.gpsimd.dma_start(b2row[:], b2d[l:l + 1, :])

                # w1s = g @ W1 ; cvec = b @ W1 + b1  (two [1,H] rows)
                w1s_row = sp.tile([1, H], BF16, tag="w1s_row", bufs=2)
                cvec_row = sp.tile([1, H], BF16, tag="cvec_row", bufs=2)
                for half in range(2):
                    psg = psB.tile([1, 512], F32, tag="small")
                    for k in range(KH):
                        nc.tensor.matmul(
                            psg[:], glb_bf[:, 2 * (KH * l + k):2 * (KH * l + k) + 1],
                            w1[k][:, half * 512:(half + 1) * 512],
                            start=(k == 0), stop=(k == KH - 1))
                    nc.scalar.activation(w1s_row[0:1, half * 512:(half + 1) * 512],
                                         psg[:], AF.Copy)
                    psb_ = psB.tile([1, 512], F32, tag="small")
                    for k in range(KH):
                        nc.tensor.matmul(
                            psb_[:],
                            glb_bf[:, 2 * (KH * l + k) + 1:2 * (KH * l + k) + 2],
                            w1[k][:, half * 512:(half + 1) * 512],
                            start=(k == 0), stop=(k == KH - 1))
                    nc.vector.tensor_tensor(
                        cvec_row[0:1, half * 512:(half + 1) * 512], psb_[:],
                        b1row[:, half * 512:(half + 1) * 512], op=ALU.add)
                # W1 <- g * W1 (in place)
                for k in range(KH):
                    nc.vector.tensor_scalar(
                        w1[k][:], w1[k][:],
                        glb_f[:, 2 * (KH * l + k):2 * (KH * l + k) + 1], None,
                        op0=ALU.mult)

                for c in range(CH):
                    q_b, negm_row, s_row = emit_stats(c, EPS_LN)
                    r_t = []
                    for n in range(KH):
                        psZ = psA.tile([128, CW], F32, tag="ps")
                        for k in range(KH):
                            nc.tensor.matmul(psZ[:],
                                             w1[k][:, n * 128:(n + 1) * 128],
                                             x[k][c][:], start=(k == 0),
                                             stop=False)
                        nc.tensor.matmul(psZ[:],
                                         w1s_row[0:1, n * 128:(n + 1) * 128],
                                         negm_row[:], start=False, stop=False)
                        nc.tensor.matmul(psZ[:],
                                         cvec_row[0:1, n * 128:(n + 1) * 128],
                                         s_row[:], start=False, stop=True)
                        r = rp.tile([128, CW], BF16, tag=f"r_{n}")
                        nc.scalar.activation(r[:], psZ[:], AF.Relu)
                        r_t.append(r)
                    for n2 in range(KH):
                        psY = psA.tile([128, CW], F32, tag="ps")
                        for n in range(KH):
                            nc.tensor.matmul(psY[:],
                                             w2[n][:, n2 * 128:(n2 + 1) * 128],
                                             r_t[n][:], start=(n == 0),
                                             stop=False)
                        nc.tensor.matmul(psY[:],
                                         b2row[:, n2 * 128:(n2 + 1) * 128],
                                         s_row[:], start=False, stop=True)
                        t = ap.tile([128, CW], BF16, tag="t")
                        nc.vector.tensor_tensor(t[:], psY[:], q_b[:],
                                                op=ALU.mult)
                        nc.vector.tensor_tensor(x[n2][c][:], x[n2][c][:], t[:],
                                                op=ALU.add)

            # ---------------- heads ----------------
            wh = []
            for k in range(KH):
                w = wp.tile([128, 2 * A], BF16, tag=f"wh_{k}", bufs=1)
                nc.gpsimd.dma_start(w[:, 0:A], Wmud[k * 128:(k + 1) * 128, :])
                nc.gpsimd.dma_start(w[:, A:2 * A], Wstdd[k * 128:(k + 1) * 128, :])
                wh.append(w)
            whs_row = sp.tile([1, 2 * A], BF16, tag="whs_row", bufs=1)
            cvech_row = sp.tile([1, 2 * A], BF16, tag="cvech_row", bufs=1)
            pshg = psB.tile([1, 512], F32, tag="small")
            for k in range(KH):
                nc.tensor.matmul(pshg[:, 0:2 * A], pglb_bf[:, 2 * k:2 * k + 1],
                                 wh[k][:], start=(k == 0), stop=(k == KH - 1))
            nc.scalar.activation(whs_row[:], pshg[:, 0:2 * A], AF.Copy)
            pshb = psB.tile([1, 512], F32, tag="small")
            for k in range(KH):
                nc.tensor.matmul(pshb[:, 0:2 * A], pglb_bf[:, 2 * k + 1:2 * k + 2],
                                 wh[k][:], start=(k == 0), stop=(k == KH - 1))
            nc.vector.tensor_tensor(cvech_row[:], pshb[:, 0:2 * A], bhead[:],
                                    op=ALU.add)

            for c in range(CH):
                q_b, negm_row, s_row = emit_stats(c, EPS_LN)
                for j in range(4):
                    # per-row 1/std as a column: transpose a q_b block
                    pst = psT.tile([128, 128], F32, tag="pst")
                    nc.tensor.transpose(pst[:], q_b[:, j * 128:(j + 1) * 128],
                                        ident[:])
                    qcol = sp.tile([128, 1], F32, tag="qcol")
                    nc.vector.tensor_copy(qcol[:], pst[:, 0:1])

                    psH = psA.tile([128, 2 * A], F32, tag="ps")
                    for k in range(KH):
                        nc.tensor.matmul(psH[:],
                                         x[k][c][:, j * 128:(j + 1) * 128],
                                         wh[k][:], start=(k == 0), stop=False)
                    nc.tensor.matmul(psH[:],
                                     negm_row[0:1, j * 128:(j + 1) * 128],
                                     whs_row[:], start=False, stop=False)
                    nc.tensor.matmul(psH[:],
                                     s_row[0:1, j * 128:(j + 1) * 128],
                                     cvech_row[:], start=False, stop=True)
                    outt = ap.tile([128, 2 * A], F32, tag="outt")
                    nc.vector.tensor_scalar(outt[:, 0:A], psH[:, 0:A], qcol[:],
                                            -5.0, op0=ALU.mult, op1=ALU.max)
                    nc.vector.tensor_scalar_min(outt[:, 0:A], outt[:, 0:A], 5.0)
                    nc.vector.tensor_scalar(outt[:, A:2 * A], psH[:, A:2 * A],
                                            qcol[:], 1.0, op0=ALU.mult,
                                            op1=ALU.min)
                    nc.vector.tensor_scalar_max(outt[:, A:2 * A],
                                                outt[:, A:2 * A], -5.0)
                    nc.scalar.activation(outt[:, A:2 * A], outt[:, A:2 * A],
                                         AF.Exp)
                    nc.sync.dma_start(
                        outd[(c * 4 + j) * 128:(c * 4 + j + 1) * 128, :],
                        outt[:])

    nc.compile()
    return nc


def _get_compiled():
    global _COMPILED
    if _COMPILED is None:
        _COMPILED = _build()
    return _COMPILED


def kernel(**inputs):
    nc = _get_compiled()
    f = lambda k: np.ascontiguousarray(np.asarray(inputs[k], dtype=np.float32))
    shared = {
        "W_in": f("W_in"),
        "b_in": f("b_in").reshape(1, H),
        "ln_g": f("ln_g"),
        "ln_b": f("ln_b"),
        "W1": f("W1"),
        "b1": f("b1"),
        "W2": f("W2"),
        "b2": f("b2"),
        "post_g": f("post_g").reshape(1, H),
        "post_b": f("post_b").reshape(1, H),
        "Wmu": f("Wmu"),
        "bmu": f("bmu").reshape(1, A),
        "Wstd": f("Wstd"),
        "bstd": f("bstd").reshape(1, A),
    }
    state = f("state")
    in_maps = []
    for i in range(NCORES):
        m = dict(shared)
        m["state"] = state[i * R:(i + 1) * R]
        in_maps.append(m)
    res = run_bass_kernel_spmd(nc, in_maps, core_ids=list(range(NCORES)))
    global LAST_RESULT
    LAST_RESULT = res
    full = np.concatenate([res.results[i]["out"] for i in range(NCORES)], axis=0)
    return full[:, :A].copy(), full[:, A:].copy()


LAST_RESULT = None


# revision 15
# speedup vs baseline: 1.1611x; 1.1611x over previous
"""Trainium2 Bass kernel for nn_Actor (RSNorm -> Linear -> 4x residual LN-MLP
blocks -> post-LN -> clipped mu/std heads), data-parallel over batch on 8
NeuronCores.

Strategy:
- Shard batch B=16384 into 8x2048 rows; weights replicated per core.
- RSNorm (Welford scan over batch) == population mean/var over batch; computed
  via per-shard bn_stats merged across cores with a tiny (4KB) AllReduce.
- All norms are folded into the adjacent matmuls: per-feature affine goes into
  the weight matrix, per-row (mean, std) corrections enter the PSUM
  accumulation as rank-2 matmuls, and the per-row 1/std scale commutes with
  ReLU so it is applied once on the residual update.
- Activations live feature-major ([feat partitions x row free]) so the whole
  residual trunk needs zero transposes; the heads flip back to row-major by
  using the activation tiles as the stationary matmul operand.
- Matmul compute in bf16 (fp32 PSUM accumulate); residual stream stored bf16.
"""

import sys

if "/opt/trn_rl_repo" not in sys.path:
    sys.path.insert(0, "/opt/trn_rl_repo")

import numpy as np

import concourse.bass as bass
import concourse.bacc as bacc
import concourse.mybir as mybir
from concourse import tile
from concourse.bass_utils import run_bass_kernel_spmd

# bass_utils imports antenv.axon_hooks when tracing is requested via
# BASS_TRACE; provide a no-op fallback module when the image lacks it.
try:
    import antenv.axon_hooks  # noqa: F401
except Exception:
    try:
        import types as _types
        import antenv as _antenv

        _m = _types.ModuleType("antenv.axon_hooks")
        _m.get_axon_ntff_profile_hook = lambda: None
        _m.set_axon_ntff_profile_hook = lambda h: None
        _antenv.axon_hooks = _m
        sys.modules["antenv.axon_hooks"] = _m
    except Exception:
        pass

F32 = mybir.dt.float32
BF16 = mybir.dt.bfloat16
AF = mybir.ActivationFunctionType
ALU = mybir.AluOpType

B, DIN, H, A, L = 16384, 512, 1024, 128, 4
NCORES = 8
R = B // NCORES          # 2048 rows per core
CH = 4                   # chunks per core
CW = R // CH             # 512 rows per chunk
KD = DIN // 128          # 4 k-tiles of the input dim
KH = H // 128            # 8 k-tiles of the hidden dim
EPS_RS = 1e-5
EPS_LN = 1e-5

_COMPILED = {}


def _build(fast):
    """fast=True assumes ln_g/post_g == 1 and every bias == 0 (the
    distributions pinned by the problem spec); kernel() verifies before
    dispatching here."""
    nc = bacc.Bacc("TRN2", target_bir_lowering=False, debug=False,
                   num_devices=NCORES)

    stated = nc.dram_tensor("state", [R, DIN], F32, kind="ExternalInput")
    W_ind = nc.dram_tensor("W_in", [DIN, H], F32, kind="ExternalInput")
    b_ind = nc.dram_tensor("b_in", [1, H], F32, kind="ExternalInput")
    ln_gd = nc.dram_tensor("ln_g", [L, H], F32, kind="ExternalInput")
    ln_bd = nc.dram_tensor("ln_b", [L, H], F32, kind="ExternalInput")
    W1d = nc.dram_tensor("W1", [L, H, H], F32, kind="ExternalInput")
    b1d = nc.dram_tensor("b1", [L, H], F32, kind="ExternalInput")
    W2d = nc.dram_tensor("W2", [L, H, H], F32, kind="ExternalInput")
    b2d = nc.dram_tensor("b2", [L, H], F32, kind="ExternalInput")
    post_gd = nc.dram_tensor("post_g", [1, H], F32, kind="ExternalInput")
    post_bd = nc.dram_tensor("post_b", [1, H], F32, kind="ExternalInput")
    Wmud = nc.dram_tensor("Wmu", [H, A], F32, kind="ExternalInput")
    bmud = nc.dram_tensor("bmu", [1, A], F32, kind="ExternalInput")
    Wstdd = nc.dram_tensor("Wstd", [H, A], F32, kind="ExternalInput")
    bstdd = nc.dram_tensor("bstd", [1, A], F32, kind="ExternalInput")
    outd = nc.dram_tensor("out", [R, 2 * A], F32, kind="ExternalOutput")

    identd = nc.inline_tensor(np.eye(128, dtype=np.float32), name="ident")

    # register 1e-5 as a const AP so activation(bias=eps) resolves
    eps_t = nc.alloc_sbuf_tensor("const-eps", [128, 1], F32)
    nc.gpsimd.memset(eps_t.ap(), EPS_LN)
    nc.const_aps.aps[(F32, EPS_LN)] = eps_t.ap()
    nc.all_engine_barrier()

    with tile.TileContext(nc) as tc:
        with (
            tc.tile_pool(name="const", bufs=1) as cp,
            tc.tile_pool(name="xp", bufs=1) as xp,
            tc.tile_pool(name="wp", bufs=2) as wp,
            tc.tile_pool(name="ap", bufs=3) as ap,
            tc.tile_pool(name="rp", bufs=2) as rp,
            tc.tile_pool(name="sp", bufs=4) as sp,
            tc.tile_pool(name="psA", bufs=6, space="PSUM") as psA,
            tc.tile_pool(name="psB", bufs=2, space="PSUM") as psB,
            tc.tile_pool(name="dp", bufs=1, space="DRAM") as dp,
        ):
            # ---------------- constants ----------------
            ones128 = cp.tile([128, 128], BF16)
            nc.vector.memset(ones128[:], 1.0)
            onesrow = cp.tile([1, CW], BF16)
            nc.vector.memset(onesrow[:], 1.0)
            ident = cp.tile([128, 128], F32)
            nc.sync.dma_start(ident[:], identd[:])

            if not fast:
                # ln_g/ln_b interleaved column tiles: [128, L*KH*2]
                glb_bf = cp.tile([128, L * KH * 2], BF16)
                glb_f = cp.tile([128, L * KH * 2], F32)
                for src, off in ((ln_gd, 0), (ln_bd, 1)):
                    view = src[:].rearrange("l (k p) -> p (l k)", p=128)
                    dst_bf = glb_bf[:].rearrange("p (lk two) -> p lk two", two=2)
                    dst_f = glb_f[:].rearrange("p (lk two) -> p lk two", two=2)
                    nc.gpsimd.dma_start(dst_bf[:, :, off], view)
                    nc.gpsimd.dma_start(dst_f[:, :, off], view)
                # post_g/post_b column tiles: [128, KH*2]
                pglb_bf = cp.tile([128, KH * 2], BF16)
                for src, off in ((post_gd, 0), (post_bd, 1)):
                    view = src[:].rearrange("o (k p) -> p (o k)", p=128)
                    dst = pglb_bf[:].rearrange("p (k two) -> p k two", two=2)
                    nc.gpsimd.dma_start(dst[:, :, off], view)
                # bias rows
                b_in_row = cp.tile([1, H], F32)
                nc.sync.dma_start(b_in_row[:], b_ind[:])
                bhead = cp.tile([1, 2 * A], F32)
                nc.sync.dma_start(bhead[:, 0:A], bmud[:])
                nc.sync.dma_start(bhead[:, A:2 * A], bstdd[:])

            # ---------------- stage A: state load + transpose ----------------
            xt = [[xp.tile([128, CW], BF16, tag=f"xt_{k}_{c}", name=f"xt_{k}_{c}")
                   for c in range(CH)] for k in range(KD)]
            srows = []
            for c in range(CH):
                for j in range(4):
                    srow = ap.tile([128, DIN], BF16, tag=f"srow{(c*4+j) % 4}",
                                   bufs=1, name=f"srow_{c}_{j}")
                    nc.gpsimd.dma_start(
                        srow[:], stated[(c * 4 + j) * 128:(c * 4 + j + 1) * 128, :])
                    srows.append((c, j, srow))
            for idx, (c, j, srow) in enumerate(srows):
                for k in range(KD):
                    nc.sync.dma_start_transpose(
                        xt[k][c][:, j * 128:(j + 1) * 128],
                        srow[:, k * 128:(k + 1) * 128])

            # ---------------- rsnorm stats + allreduce ----------------
            allin = sp.tile([128, KD * 2], F32, tag="allin", bufs=1)
            for k in range(KD):
                bnbuf = sp.tile([128, CH * 6], F32, tag=f"bn_{k}", bufs=1)
                for c in range(CH):
                    nc.vector.bn_stats(bnbuf[:, c * 6:(c + 1) * 6], xt[k][c][:])
                aggr = sp.tile([128, 2], F32, tag=f"aggr_{k}", bufs=1)
                nc.vector.bn_aggr(
                    aggr[:], bnbuf[:].rearrange("p (c s) -> p c s", s=6))
                # sum = mean * R ; sumsq = (var + mean^2) * R
                nc.scalar.activation(allin[:, 2 * k:2 * k + 1], aggr[:, 0:1],
                                     AF.Copy, scale=float(R))
                t1 = sp.tile([128, 1], F32, tag="t1")
                nc.vector.tensor_tensor(t1[:], aggr[:, 0:1], aggr[:, 0:1],
                                        op=ALU.mult)
                nc.vector.tensor_tensor(t1[:], t1[:], aggr[:, 1:2], op=ALU.add)
                nc.scalar.activation(allin[:, 2 * k + 1:2 * k + 2], t1[:],
                                     AF.Copy, scale=float(R))
            cc_in = dp.tile([128, KD * 2], F32)
            cc_out = dp.tile([128, KD * 2], F32, addr_space="Shared")
            nc.gpsimd.dma_start(cc_in[:], allin[:])
            nc.gpsimd.collective_compute(
                "AllReduce", ALU.add,
                replica_groups=[list(range(NCORES))],
                ins=[cc_in[:].opt()], outs=[cc_out[:].opt()])
            allout = sp.tile([128, KD * 2], F32, tag="allout", bufs=1)
            nc.gpsimd.dma_start(allout[:], cc_out[:])

            # per-feature fold factors for W_in
            a_col = []
            c_col = []
            for k in range(KD):
                muk = sp.tile([128, 1], F32, tag=f"muk_{k}", bufs=1)
                nc.scalar.activation(muk[:], allout[:, 2 * k:2 * k + 1],
                                     AF.Copy, scale=1.0 / B)
                var = sp.tile([128, 1], F32, tag="var1")
                nc.scalar.activation(var[:], allout[:, 2 * k + 1:2 * k + 2],
                                     AF.Copy, scale=1.0 / B)
                msq = sp.tile([128, 1], F32, tag="msq1")
                nc.vector.tensor_tensor(msq[:], muk[:], muk[:], op=ALU.mult)
                nc.vector.tensor_tensor(var[:], var[:], msq[:], op=ALU.subtract)
                nc.vector.tensor_scalar_max(var[:], var[:], 0.001)
                ak = sp.tile([128, 1], F32, tag=f"ak_{k}", bufs=1)
                nc.scalar.activation(ak[:], var[:], AF.Abs_reciprocal_sqrt,
                                     bias=EPS_RS)
                mak = sp.tile([128, 1], F32, tag="mak")
                nc.vector.tensor_tensor(mak[:], muk[:], ak[:], op=ALU.mult)
                ck = sp.tile([128, 1], BF16, tag=f"ck_{k}", bufs=1)
                nc.scalar.activation(ck[:], mak[:], AF.Copy, scale=-1.0)
                a_col.append(ak)
                c_col.append(ck)

            # ---------------- W_in fold + x1 ----------------
            w_in = []
            for k in range(KD):
                w = wp.tile([128, H], BF16, tag=f"win_{k}", bufs=1)
                nc.gpsimd.dma_start(w[:], W_ind[k * 128:(k + 1) * 128, :])
                w_in.append(w)
            # dvec = c @ W_in + b_in
            dvec = sp.tile([1, H], BF16, tag="dvec", bufs=1)
            for half in range(2):
                psd = psB.tile([2, 512], F32, tag="small")
                for k in range(KD):
                    nc.tensor.matmul(psd[0:1, :], c_col[k][:],
                                     w_in[k][:, half * 512:(half + 1) * 512],
                                     start=(k == 0), stop=(k == KD - 1))
                if fast:
                    nc.scalar.activation(dvec[:, half * 512:(half + 1) * 512],
                                         psd[0:1, :], AF.Copy)
                else:
                    nc.vector.tensor_tensor(
                        dvec[:, half * 512:(half + 1) * 512], psd[0:1, :],
                        b_in_row[:, half * 512:(half + 1) * 512], op=ALU.add)
            # W_in <- a * W_in (in place, after dvec matmuls)
            for k in range(KD):
                nc.vector.tensor_scalar(w_in[k][:], w_in[k][:], a_col[k][:],
                                        None, op0=ALU.mult)

            x = [[xp.tile([128, CW], BF16, tag=f"x_{n}_{c}", name=f"x_{n}_{c}")
                  for c in range(CH)] for n in range(KH)]
            for c0 in range(0, CH, 2):
                c1 = c0 + 1
                for n in range(KH):
                    ps0 = psA.tile([128, CW], F32, tag="ps", name="ps0")
                    ps1 = psA.tile([128, CW], F32, tag="ps", name="ps1")
                    for k in range(KD):
                        nc.tensor.matmul(ps0[:], w_in[k][:, n * 128:(n + 1) * 128],
                                         xt[k][c0][:], start=(k == 0), stop=False)
                        nc.tensor.matmul(ps1[:], w_in[k][:, n * 128:(n + 1) * 128],
                                         xt[k][c1][:], start=(k == 0), stop=False)
                    nc.tensor.matmul(ps0[:], dvec[:, n * 128:(n + 1) * 128],
                                     onesrow[:], start=False, stop=True)
                    nc.tensor.matmul(ps1[:], dvec[:, n * 128:(n + 1) * 128],
                                     onesrow[:], start=False, stop=True)
                    nc.scalar.activation(x[n][c0][:], ps0[:], AF.Copy)
                    nc.scalar.activation(x[n][c1][:], ps1[:], AF.Copy)

            # ---------------- helper: per-chunk-pair row stats ----------------
            def stats_chain(pss, psq, eps):
                negm_row = sp.tile([1, CW], BF16, tag="negm", bufs=4,
                                   name="negm_row")
                nc.scalar.activation(negm_row[:], pss[0:1, :], AF.Copy,
                                     scale=-1.0 / H)
                m_b = ap.tile([128, CW], F32, tag="m_b", bufs=2, name="m_b")
                nc.scalar.activation(m_b[:], pss[:], AF.Copy, scale=1.0 / H)
                nc.vector.tensor_tensor(m_b[:], m_b[:], m_b[:], op=ALU.mult)
                var = ap.tile([128, CW], F32, tag="varb", bufs=2, name="var")
                nc.vector.scalar_tensor_tensor(var[:], psq[:], 1.0 / H, m_b[:],
                                               op0=ALU.mult, op1=ALU.subtract)
                q_b = ap.tile([128, CW], F32, tag="qb", bufs=3, name="q_b")
                nc.scalar.activation(q_b[:], var[:], AF.Abs_reciprocal_sqrt,
                                     bias=eps)
                if fast:
                    return q_b, negm_row, None
                vpe = ap.tile([1, CW], F32, tag="vpe", bufs=1, name="vpe")
                nc.vector.tensor_scalar_add(vpe[:], var[0:1, :], eps)
                s_row = sp.tile([1, CW], BF16, tag="s_row", bufs=2,
                                name="s_row")
                nc.vector.tensor_tensor(s_row[:], vpe[:], q_b[0:1, :],
                                        op=ALU.mult)
                return q_b, negm_row, s_row

            def emit_stats_pair(c0, c1, eps):
                pss0 = psA.tile([128, CW], F32, tag="ps", name="pss0")
                psq0 = psA.tile([128, CW], F32, tag="ps", name="psq0")
                pss1 = psA.tile([128, CW], F32, tag="ps", name="pss1")
                psq1 = psA.tile([128, CW], F32, tag="ps", name="psq1")
                for k in range(KH):
                    sq0 = ap.tile([128, CW], BF16, tag="sq", name="sq0")
                    nc.scalar.activation(sq0[:], x[k][c0][:], AF.Square)
                    sq1 = ap.tile([128, CW], BF16, tag="sq", name="sq1")
                    nc.scalar.activation(sq1[:], x[k][c1][:], AF.Square)
                    nc.tensor.matmul(pss0[:], ones128[:], x[k][c0][:],
                                     start=(k == 0), stop=(k == KH - 1))
                    nc.tensor.matmul(psq0[:], ones128[:], sq0[:],
                                     start=(k == 0), stop=(k == KH - 1))
                    nc.tensor.matmul(pss1[:], ones128[:], x[k][c1][:],
                                     start=(k == 0), stop=(k == KH - 1))
                    nc.tensor.matmul(psq1[:], ones128[:], sq1[:],
                                     start=(k == 0), stop=(k == KH - 1))
                st0 = stats_chain(pss0, psq0, eps)
                st1 = stats_chain(pss1, psq1, eps)
                return st0, st1

            # ---------------- blocks ----------------
            for l in range(L):
                w1 = []
                w2 = []
                for k in range(KH):
                    w = wp.tile([128, H], BF16, tag=f"w1_{k}")
                    nc.gpsimd.dma_start(w[:], W1d[l, k * 128:(k + 1) * 128, :])
                    w1.append(w)
                for k in range(KH):
                    w = wp.tile([128, H], BF16, tag=f"w2_{k}")
                    nc.gpsimd.dma_start(w[:], W2d[l, k * 128:(k + 1) * 128, :])
                    w2.append(w)
                w1s_row = sp.tile([1, H], BF16, tag="w1s_row", bufs=2)
                if not fast:
                    b1row = sp.tile([1, H], F32, tag="b1row", bufs=1)
                    nc.sync.dma_start(b1row[:], b1d[l:l + 1, :])
                    b2row = sp.tile([1, H], BF16, tag="b2row", bufs=1)
                    nc.gpsimd.dma_start(b2row[:], b2d[l:l + 1, :])
                    cvec_row = sp.tile([1, H], BF16, tag="cvec_row", bufs=2)
                for half in range(2):
                    g_lhs = (ones128[:, 0:1] if fast else
                             glb_bf[:, 2 * (KH * l):2 * (KH * l) + 1])
                    psg = psB.tile([1, 512], F32, tag="small")
                    for k in range(KH):
                        nc.tensor.matmul(
                            psg[:],
                            ones128[:, 0:1] if fast else
                            glb_bf[:, 2 * (KH * l + k):2 * (KH * l + k) + 1],
                            w1[k][:, half * 512:(half + 1) * 512],
                            start=(k == 0), stop=(k == KH - 1))
                    nc.scalar.activation(w1s_row[0:1, half * 512:(half + 1) * 512],
                                         psg[:], AF.Copy)
                    if not fast:
                        psb_ = psB.tile([1, 512], F32, tag="small")
                        for k in range(KH):
                            nc.tensor.matmul(
                                psb_[:],
                                glb_bf[:, 2 * (KH * l + k) + 1:2 * (KH * l + k) + 2],
                                w1[k][:, half * 512:(half + 1) * 512],
                                start=(k == 0), stop=(k == KH - 1))
                        nc.vector.tensor_tensor(
                            cvec_row[0:1, half * 512:(half + 1) * 512], psb_[:],
                            b1row[:, half * 512:(half + 1) * 512], op=ALU.add)
                if not fast:
                    # W1 <- g * W1 (in place)
                    for k in range(KH):
                        nc.vector.tensor_scalar(
                            w1[k][:], w1[k][:],
                            glb_f[:, 2 * (KH * l + k):2 * (KH * l + k) + 1],
                            None, op0=ALU.mult)

                for c0 in range(0, CH, 2):
                    c1 = c0 + 1
                    (q0, negm0, srow0), (q1, negm1, srow1) = \
                        emit_stats_pair(c0, c1, EPS_LN)
                    r0_t = []
                    r1_t = []
                    for n in range(KH):
                        psZ0 = psA.tile([128, CW], F32, tag="ps", name="psZ0")
                        psZ1 = psA.tile([128, CW], F32, tag="ps", name="psZ1")
                        for k in range(KH):
                            nc.tensor.matmul(psZ0[:],
                                             w1[k][:, n * 128:(n + 1) * 128],
                                             x[k][c0][:], start=(k == 0),
                                             stop=False)
                            nc.tensor.matmul(psZ1[:],
                                             w1[k][:, n * 128:(n + 1) * 128],
                                             x[k][c1][:], start=(k == 0),
                                             stop=False)
                        nc.tensor.matmul(psZ0[:],
                                         w1s_row[0:1, n * 128:(n + 1) * 128],
                                         negm0[:], start=False, stop=fast)
                        nc.tensor.matmul(psZ1[:],
                                         w1s_row[0:1, n * 128:(n + 1) * 128],
                                         negm1[:], start=False, stop=fast)
                        if not fast:
                            nc.tensor.matmul(
                                psZ0[:], cvec_row[0:1, n * 128:(n + 1) * 128],
                                srow0[:], start=False, stop=True)
                            nc.tensor.matmul(
                                psZ1[:], cvec_row[0:1, n * 128:(n + 1) * 128],
                                srow1[:], start=False, stop=True)
                        r0 = rp.tile([128, CW], BF16, tag=f"r0_{n}", bufs=1,
                                     name=f"r0_{n}")
                        nc.scalar.activation(r0[:], psZ0[:], AF.Relu)
                        r0_t.append(r0)
                        r1 = rp.tile([128, CW], BF16, tag=f"r1_{n}", bufs=1,
                                     name=f"r1_{n}")
                        nc.scalar.activation(r1[:], psZ1[:], AF.Relu)
                        r1_t.append(r1)
                    for n2 in range(KH):
                        psY0 = psA.tile([128, CW], F32, tag="ps", name="psY0")
                        psY1 = psA.tile([128, CW], F32, tag="ps", name="psY1")
                        for n in range(KH):
                            nc.tensor.matmul(psY0[:],
                                             w2[n][:, n2 * 128:(n2 + 1) * 128],
                                             r0_t[n][:], start=(n == 0),
                                             stop=(fast and n == KH - 1))
                            nc.tensor.matmul(psY1[:],
                                             w2[n][:, n2 * 128:(n2 + 1) * 128],
                                             r1_t[n][:], start=(n == 0),
                                             stop=(fast and n == KH - 1))
                        if not fast:
                            nc.tensor.matmul(psY0[:],
                                             b2row[:, n2 * 128:(n2 + 1) * 128],
                                             srow0[:], start=False, stop=True)
                            nc.tensor.matmul(psY1[:],
                                             b2row[:, n2 * 128:(n2 + 1) * 128],
                                             srow1[:], start=False, stop=True)
                        t0 = ap.tile([128, CW], BF16, tag="t", name="t0")
                        nc.vector.tensor_tensor(t0[:], psY0[:], q0[:],
                                                op=ALU.mult)
                        nc.vector.tensor_tensor(x[n2][c0][:], x[n2][c0][:],
                                                t0[:], op=ALU.add)
                        t1 = ap.tile([128, CW], BF16, tag="t", name="t1")
                        nc.vector.tensor_tensor(t1[:], psY1[:], q1[:],
                                                op=ALU.mult)
                        nc.vector.tensor_tensor(x[n2][c1][:], x[n2][c1][:],
                                                t1[:], op=ALU.add)

            # ---------------- heads ----------------
            wh = []
            for k in range(KH):
                w = wp.tile([128, 2 * A], BF16, tag=f"wh_{k}", bufs=1)
                nc.gpsimd.dma_start(w[:, 0:A], Wmud[k * 128:(k + 1) * 128, :])
                nc.gpsimd.dma_start(w[:, A:2 * A], Wstdd[k * 128:(k + 1) * 128, :])
                wh.append(w)
            whs_row = sp.tile([1, 2 * A], BF16, tag="whs_row", bufs=1)
            pshg = psB.tile([1, 512], F32, tag="small")
            for k in range(KH):
                nc.tensor.matmul(pshg[:, 0:2 * A],
                                 ones128[:, 0:1] if fast else
                                 pglb_bf[:, 2 * k:2 * k + 1],
                                 wh[k][:], start=(k == 0), stop=(k == KH - 1))
            nc.scalar.activation(whs_row[:], pshg[:, 0:2 * A], AF.Copy)
            if not fast:
                cvech_row = sp.tile([1, 2 * A], BF16, tag="cvech_row", bufs=1)
                pshb = psB.tile([1, 512], F32, tag="small")
                for k in range(KH):
                    nc.tensor.matmul(pshb[:, 0:2 * A],
                                     pglb_bf[:, 2 * k + 1:2 * k + 2],
                                     wh[k][:], start=(k == 0), stop=(k == KH - 1))
                nc.vector.tensor_tensor(cvech_row[:], pshb[:, 0:2 * A],
                                        bhead[:], op=ALU.add)

            for c0 in range(0, CH, 2):
                hstats = emit_stats_pair(c0, c0 + 1, EPS_LN)
                for cc in range(2):
                  c = c0 + cc
                  q_b, negm_row, s_row = hstats[cc]
                  for j in range(4):
                    # per-row 1/std as a column: transpose a q_b block
                    pst = psB.tile([128, 128], F32, tag="small", name="pst")
                    nc.tensor.transpose(pst[:], q_b[:, j * 128:(j + 1) * 128],
                                        ident[:])
                    qcol = sp.tile([128, 1], F32, tag="qcol")
                    nc.vector.tensor_copy(qcol[:], pst[:, 0:1])

                    psH = psA.tile([128, 2 * A], F32, tag="ps")
                    for k in range(KH):
                        nc.tensor.matmul(psH[:],
                                         x[k][c][:, j * 128:(j + 1) * 128],
                                         wh[k][:], start=(k == 0), stop=False)
                    nc.tensor.matmul(psH[:],
                                     negm_row[0:1, j * 128:(j + 1) * 128],
                                     whs_row[:], start=False, stop=fast)
                    if not fast:
                        nc.tensor.matmul(psH[:],
                                         s_row[0:1, j * 128:(j + 1) * 128],
                                         cvech_row[:], start=False, stop=True)
                    outt = ap.tile([128, 2 * A], F32, tag="outt")
                    nc.vector.tensor_scalar(outt[:, 0:A], psH[:, 0:A], qcol[:],
                                            -5.0, op0=ALU.mult, op1=ALU.max)
                    nc.vector.tensor_scalar_min(outt[:, 0:A], outt[:, 0:A], 5.0)
                    nc.vector.tensor_scalar(outt[:, A:2 * A], psH[:, A:2 * A],
                                            qcol[:], 1.0, op0=ALU.mult,
                                            op1=ALU.min)
                    nc.vector.tensor_scalar_max(outt[:, A:2 * A],
                                                outt[:, A:2 * A], -5.0)
                    nc.scalar.activation(outt[:, A:2 * A], outt[:, A:2 * A],
                                         AF.Exp)
                    nc.sync.dma_start(
                        outd[(c * 4 + j) * 128:(c * 4 + j + 1) * 128, :],
                        outt[:])

    nc.compile()
    return nc


def _get_compiled(fast=True):
    if fast not in _COMPILED:
        _COMPILED[fast] = _build(fast)
    return _COMPILED[fast]


def _fast_ok(inputs):
    z = lambda k: not np.any(np.asarray(inputs[k]))
    o = lambda k: np.all(np.asarray(inputs[k]) == 1.0)
    return (z("b_in") and z("ln_b") and z("b1") and z("b2") and z("post_b")
            and z("bmu") and z("bstd") and o("ln_g") and o("post_g"))


def kernel(**inputs):
    nc = _get_compiled(fast=_fast_ok(inputs))
    f = lambda k: np.ascontiguousarray(np.asarray(inputs[k], dtype=np.float32))
    shared = {
        "W_in": f("W_in"),
        "b_in": f("b_in").reshape(1, H),
        "ln_g": f("ln_g"),
        "ln_b": f("ln_b"),
        "W1": f("W1"),
        "b1": f("b1"),
        "W2": f("W2"),
        "b2": f("b2"),
        "post_g": f("post_g").reshape(1, H),
        "post_b": f("post_b").reshape(1, H),
        "Wmu": f("Wmu"),
        "bmu": f("bmu").reshape(1, A),
        "Wstd": f("Wstd"),
        "bstd": f("bstd").reshape(1, A),
    }
    state = f("state")
    in_maps = []
    for i in range(NCORES):
        m = dict(shared)
        m["state"] = state[i * R:(i + 1) * R]
        in_maps.append(m)
    res = run_bass_kernel_spmd(nc, in_maps, core_ids=list(range(NCORES)))
    global LAST_RESULT
    LAST_RESULT = res
    full = np.concatenate([res.results[i]["out"] for i in range(NCORES)], axis=0)
    return full[:, :A].copy(), full[:, A:].copy()


LAST_RESULT = None


# revision 19
# speedup vs baseline: 1.3443x; 1.1578x over previous
"""Trainium2 Bass kernel for nn_Actor (RSNorm -> Linear -> 4x residual LN-MLP
blocks -> post-LN -> clipped mu/std heads), data-parallel over batch on 8
NeuronCores.

Strategy:
- Shard batch B=16384 into 8x2048 rows; weights replicated per core.
- RSNorm (Welford scan over batch) == population mean/var over batch; computed
  via per-shard bn_stats merged across cores with a tiny (4KB) AllReduce.
- All norms are folded into the adjacent matmuls: per-feature affine goes into
  the weight matrix, per-row (mean, std) corrections enter the PSUM
  accumulation as rank-2 matmuls, and the per-row 1/std scale commutes with
  ReLU so it is applied once on the residual update.
- Activations live feature-major ([feat partitions x row free]) so the whole
  residual trunk needs zero transposes; the heads flip back to row-major by
  using the activation tiles as the stationary matmul operand.
- Matmul compute in bf16 (fp32 PSUM accumulate); residual stream stored bf16.
"""

import sys

if "/opt/trn_rl_repo" not in sys.path:
    sys.path.insert(0, "/opt/trn_rl_repo")

import numpy as np

import concourse.bass as bass
import concourse.bacc as bacc
import concourse.mybir as mybir
from concourse import tile
from concourse.bass_utils import run_bass_kernel_spmd

# bass_utils imports antenv.axon_hooks when tracing is requested via
# BASS_TRACE; provide a no-op fallback module when the image lacks it.
try:
    import antenv.axon_hooks  # noqa: F401
except Exception:
    try:
        import types as _types
        import antenv as _antenv

        _m = _types.ModuleType("antenv.axon_hooks")
        _m.get_axon_ntff_profile_hook = lambda: None
        _m.set_axon_ntff_profile_hook = lambda h: None
        _antenv.axon_hooks = _m
        sys.modules["antenv.axon_hooks"] = _m
    except Exception:
        pass

F32 = mybir.dt.float32
BF16 = mybir.dt.bfloat16
AF = mybir.ActivationFunctionType
ALU = mybir.AluOpType

B, DIN, H, A, L = 16384, 512, 1024, 128, 4
NCORES = 8
R = B // NCORES          # 2048 rows per core
CH = 4                   # chunks per core
CW = R // CH             # 512 rows per chunk
KD = DIN // 128          # 4 k-tiles of the input dim
KH = H // 128            # 8 k-tiles of the hidden dim
EPS_RS = 1e-5
EPS_LN = 1e-5

_COMPILED = {}


def _build(fast):
    """fast=True assumes ln_g/post_g == 1 and every bias == 0 (the
    distributions pinned by the problem spec); kernel() verifies before
    dispatching here."""
    nc = bacc.Bacc("TRN2", target_bir_lowering=False, debug=False,
                   num_devices=NCORES)

    stated = nc.dram_tensor("state", [R, DIN], F32, kind="ExternalInput")
    W_ind = nc.dram_tensor("W_in", [DIN, H], F32, kind="ExternalInput")
    b_ind = nc.dram_tensor("b_in", [1, H], F32, kind="ExternalInput")
    ln_gd = nc.dram_tensor("ln_g", [L, H], F32, kind="ExternalInput")
    ln_bd = nc.dram_tensor("ln_b", [L, H], F32, kind="ExternalInput")
    W1d = nc.dram_tensor("W1", [L, H, H], F32, kind="ExternalInput")
    b1d = nc.dram_tensor("b1", [L, H], F32, kind="ExternalInput")
    W2d = nc.dram_tensor("W2", [L, H, H], F32, kind="ExternalInput")
    b2d = nc.dram_tensor("b2", [L, H], F32, kind="ExternalInput")
    post_gd = nc.dram_tensor("post_g", [1, H], F32, kind="ExternalInput")
    post_bd = nc.dram_tensor("post_b", [1, H], F32, kind="ExternalInput")
    Wmud = nc.dram_tensor("Wmu", [H, A], F32, kind="ExternalInput")
    bmud = nc.dram_tensor("bmu", [1, A], F32, kind="ExternalInput")
    Wstdd = nc.dram_tensor("Wstd", [H, A], F32, kind="ExternalInput")
    bstdd = nc.dram_tensor("bstd", [1, A], F32, kind="ExternalInput")
    outd = nc.dram_tensor("out", [R, 2 * A], F32, kind="ExternalOutput")

    identd = nc.inline_tensor(np.eye(128, dtype=np.float32), name="ident")

    # register 1e-5 as a const AP so activation(bias=eps) resolves
    eps_t = nc.alloc_sbuf_tensor("const-eps", [128, 1], F32)
    nc.gpsimd.memset(eps_t.ap(), EPS_LN)
    nc.const_aps.aps[(F32, EPS_LN)] = eps_t.ap()
    nc.all_engine_barrier()

    with tile.TileContext(nc) as tc:
        with (
            tc.tile_pool(name="const", bufs=1) as cp,
            tc.tile_pool(name="xp", bufs=1) as xp,
            tc.tile_pool(name="wp", bufs=2) as wp,
            tc.tile_pool(name="ap", bufs=3) as ap,
            tc.tile_pool(name="rp", bufs=2) as rp,
            tc.tile_pool(name="sp", bufs=4) as sp,
            tc.tile_pool(name="psA", bufs=6, space="PSUM") as psA,
            tc.tile_pool(name="psB", bufs=2, space="PSUM") as psB,
            tc.tile_pool(name="dp", bufs=1, space="DRAM") as dp,
        ):
            # ---------------- constants ----------------
            ones128 = cp.tile([128, 128], BF16)
            nc.vector.memset(ones128[:], 1.0)
            onesrow = cp.tile([1, CW], BF16)
            nc.vector.memset(onesrow[:], 1.0)
            ident = cp.tile([128, 128], F32)
            nc.sync.dma_start(ident[:], identd[:])
            identb = cp.tile([128, 128], BF16)
            nc.gpsimd.dma_start(identb[:], identd[:])

            if not fast:
                # ln_g/ln_b interleaved column tiles: [128, L*KH*2]
                glb_bf = cp.tile([128, L * KH * 2], BF16)
                glb_f = cp.tile([128, L * KH * 2], F32)
                for src, off in ((ln_gd, 0), (ln_bd, 1)):
                    view = src[:].rearrange("l (k p) -> p (l k)", p=128)
                    dst_bf = glb_bf[:].rearrange("p (lk two) -> p lk two", two=2)
                    dst_f = glb_f[:].rearrange("p (lk two) -> p lk two", two=2)
                    nc.gpsimd.dma_start(dst_bf[:, :, off], view)
                    nc.gpsimd.dma_start(dst_f[:, :, off], view)
                # post_g/post_b column tiles: [128, KH*2]
                pglb_bf = cp.tile([128, KH * 2], BF16)
                for src, off in ((post_gd, 0), (post_bd, 1)):
                    view = src[:].rearrange("o (k p) -> p (o k)", p=128)
                    dst = pglb_bf[:].rearrange("p (k two) -> p k two", two=2)
                    nc.gpsimd.dma_start(dst[:, :, off], view)
                # bias rows
                b_in_row = cp.tile([1, H], F32)
                nc.sync.dma_start(b_in_row[:], b_ind[:])
                bhead = cp.tile([1, 2 * A], F32)
                nc.sync.dma_start(bhead[:, 0:A], bmud[:])
                nc.sync.dma_start(bhead[:, A:2 * A], bstdd[:])

            # ---------------- stage A: state load + transpose ----------------
            xt = [[xp.tile([128, CW], BF16, tag=f"xt_{k}_{c}", name=f"xt_{k}_{c}")
                   for c in range(CH)] for k in range(KD)]
            # state row tiles (bf16) + per-feature batch sums on the PE
            ps_rsum = psA.tile([128, DIN], F32, tag="ps", name="ps_rsum")
            ps_rsq = psA.tile([128, DIN], F32, tag="ps", name="ps_rsq")
            srows = []
            for c in range(CH):
                for j in range(4):
                    idx = c * 4 + j
                    srow = ap.tile([128, DIN], BF16, tag=f"srow{idx % 4}",
                                   bufs=1, name=f"srow_{c}_{j}")
                    nc.gpsimd.dma_start(
                        srow[:], stated[idx * 128:(idx + 1) * 128, :])
                    sqr = ap.tile([128, DIN], BF16, tag="sqr", name="sqr")
                    nc.scalar.activation(sqr[:], srow[:], AF.Square)
                    nc.tensor.matmul(ps_rsum[:], ones128[:], srow[:],
                                     start=(idx == 0), stop=(idx == CH * 4 - 1))
                    nc.tensor.matmul(ps_rsq[:], ones128[:], sqr[:],
                                     start=(idx == 0), stop=(idx == CH * 4 - 1))
                    srows.append((c, j, srow))
            # transpose state tiles on the PE (bf16, 1 cyc/row)
            for c, j, srow in srows:
                for k in range(KD):
                    pst = psA.tile([128, 128], BF16, tag="ps", name="pstr")
                    nc.tensor.transpose(pst[:], srow[:, k * 128:(k + 1) * 128],
                                        identb[:])
                    nc.vector.tensor_copy(xt[k][c][:, j * 128:(j + 1) * 128],
                                          pst[:])

            # ---------------- rsnorm allreduce ----------------
            rsum_row = sp.tile([1, DIN], F32, tag="rsum_row", bufs=1)
            nc.scalar.activation(rsum_row[:], ps_rsum[0:1, :], AF.Copy)
            rsq_row = sp.tile([1, DIN], F32, tag="rsq_row", bufs=1)
            nc.scalar.activation(rsq_row[:], ps_rsq[0:1, :], AF.Copy)
            cc_in = dp.tile([2, DIN], F32)
            cc_out = dp.tile([2, DIN], F32, addr_space="Shared")
            nc.sync.dma_start(cc_in[0:1, :], rsum_row[:])
            nc.sync.dma_start(cc_in[1:2, :], rsq_row[:])
            nc.gpsimd.collective_compute(
                "AllReduce", ALU.add,
                replica_groups=[list(range(NCORES))],
                ins=[cc_in[:].opt()], outs=[cc_out[:].opt()])
            allout = sp.tile([2, DIN], F32, tag="allout", bufs=1)
            nc.gpsimd.dma_start(allout[:], cc_out[:])

            # per-feature fold factors for W_in: transpose [2,128] stat
            # blocks into [128,2] columns, then the scalar chain
            a_col = []
            c_col = []
            for k in range(KD):
                pstc = psB.tile([128, 2], F32, tag="small", name="pstc")
                nc.tensor.transpose(pstc[:],
                                    allout[0:2, k * 128:(k + 1) * 128],
                                    ident[0:2, 0:2])
                stc = sp.tile([128, 2], F32, tag=f"stc_{k}", bufs=1)
                nc.vector.tensor_copy(stc[:], pstc[:])
                muk = sp.tile([128, 1], F32, tag=f"muk_{k}", bufs=1)
                nc.scalar.activation(muk[:], stc[:, 0:1],
                                     AF.Copy, scale=1.0 / B)
                var = sp.tile([128, 1], F32, tag="var1")
                nc.scalar.activation(var[:], stc[:, 1:2],
                                     AF.Copy, scale=1.0 / B)
                msq = sp.tile([128, 1], F32, tag="msq1")
                nc.vector.tensor_tensor(msq[:], muk[:], muk[:], op=ALU.mult)
                nc.vector.tensor_tensor(var[:], var[:], msq[:], op=ALU.subtract)
                nc.vector.tensor_scalar_max(var[:], var[:], 0.001)
                ak = sp.tile([128, 1], F32, tag=f"ak_{k}", bufs=1)
                nc.scalar.activation(ak[:], var[:], AF.Abs_reciprocal_sqrt,
                                     bias=EPS_RS)
                mak = sp.tile([128, 1], F32, tag="mak")
                nc.vector.tensor_tensor(mak[:], muk[:], ak[:], op=ALU.mult)
                ck = sp.tile([128, 1], BF16, tag=f"ck_{k}", bufs=1)
                nc.scalar.activation(ck[:], mak[:], AF.Copy, scale=-1.0)
                a_col.append(ak)
                c_col.append(ck)

            # ---------------- W_in fold + x1 ----------------
            w_in = []
            for k in range(KD):
                w = wp.tile([128, H], BF16, tag=f"win_{k}", bufs=1)
                nc.gpsimd.dma_start(w[:], W_ind[k * 128:(k + 1) * 128, :])
                w_in.append(w)
            # dvec = c @ W_in + b_in
            dvec = sp.tile([1, H], BF16, tag="dvec", bufs=1)
            for half in range(2):
                psd = psB.tile([2, 512], F32, tag="small")
                for k in range(KD):
                    nc.tensor.matmul(psd[0:1, :], c_col[k][:],
                                     w_in[k][:, half * 512:(half + 1) * 512],
                                     start=(k == 0), stop=(k == KD - 1))
                if fast:
                    nc.scalar.activation(dvec[:, half * 512:(half + 1) * 512],
                                         psd[0:1, :], AF.Copy)
                else:
                    nc.vector.tensor_tensor(
                        dvec[:, half * 512:(half + 1) * 512], psd[0:1, :],
                        b_in_row[:, half * 512:(half + 1) * 512], op=ALU.add)
            # W_in <- a * W_in (in place, after dvec matmuls)
            for k in range(KD):
                nc.vector.tensor_scalar(w_in[k][:], w_in[k][:], a_col[k][:],
                                        None, op0=ALU.mult)

            x = [[xp.tile([128, CW], BF16, tag=f"x_{n}_{c}", name=f"x_{n}_{c}")
                  for c in range(CH)] for n in range(KH)]
            for c0 in range(0, CH, 2):
                c1 = c0 + 1
                for n in range(KH):
                    ps0 = psA.tile([128, CW], F32, tag="ps", name="ps0")
                    ps1 = psA.tile([128, CW], F32, tag="ps", name="ps1")
                    for k in range(KD):
                        nc.tensor.matmul(ps0[:], w_in[k][:, n * 128:(n + 1) * 128],
                                         xt[k][c0][:], start=(k == 0), stop=False)
                        nc.tensor.matmul(ps1[:], w_in[k][:, n * 128:(n + 1) * 128],
                                         xt[k][c1][:], start=(k == 0), stop=False)
                    nc.tensor.matmul(ps0[:], dvec[:, n * 128:(n + 1) * 128],
                                     onesrow[:], start=False, stop=True)
                    nc.tensor.matmul(ps1[:], dvec[:, n * 128:(n + 1) * 128],
                                     onesrow[:], start=False, stop=True)
                    nc.scalar.activation(x[n][c0][:], ps0[:], AF.Copy)
                    nc.scalar.activation(x[n][c1][:], ps1[:], AF.Copy)

            # ---------------- helper: per-chunk-pair row stats ----------------
            def stats_chain(pss, psq, eps):
                negm_row = sp.tile([1, CW], BF16, tag="negm", bufs=3,
                                   name="negm_row")
                nc.scalar.activation(negm_row[:], pss[0:1, :], AF.Copy,
                                     scale=-1.0 / H)
                m_b = ap.tile([128, CW], F32, tag="m_b", bufs=2, name="m_b")
                nc.scalar.activation(m_b[:], pss[:], AF.Copy, scale=1.0 / H)
                nc.vector.tensor_tensor(m_b[:], m_b[:], m_b[:], op=ALU.mult)
                var = ap.tile([128, CW], F32, tag="varb", bufs=2, name="var")
                nc.vector.scalar_tensor_tensor(var[:], psq[:], 1.0 / H, m_b[:],
                                               op0=ALU.mult, op1=ALU.subtract)
                q_b = ap.tile([128, CW], F32, tag="qb", bufs=3, name="q_b")
                nc.scalar.activation(q_b[:], var[:], AF.Abs_reciprocal_sqrt,
                                     bias=eps)
                if fast:
                    return q_b, negm_row, None
                vpe = ap.tile([1, CW], F32, tag="vpe", bufs=1, name="vpe")
                nc.vector.tensor_scalar_add(vpe[:], var[0:1, :], eps)
                s_row = sp.tile([1, CW], BF16, tag="s_row", bufs=2,
                                name="s_row")
                nc.vector.tensor_tensor(s_row[:], vpe[:], q_b[0:1, :],
                                        op=ALU.mult)
                return q_b, negm_row, s_row

            def emit_stats_pair(c0, c1, eps):
                pss0 = psA.tile([128, CW], F32, tag="ps", name="pss0")
                psq0 = psA.tile([128, CW], F32, tag="ps", name="psq0")
                pss1 = psA.tile([128, CW], F32, tag="ps", name="pss1")
                psq1 = psA.tile([128, CW], F32, tag="ps", name="psq1")
                for k in range(KH):
                    sq0 = ap.tile([128, CW], BF16, tag="sq", name="sq0")
                    nc.scalar.activation(sq0[:], x[k][c0][:], AF.Square)
                    sq1 = ap.tile([128, CW], BF16, tag="sq", name="sq1")
                    nc.scalar.activation(sq1[:], x[k][c1][:], AF.Square)
                    nc.tensor.matmul(pss0[:], ones128[:], x[k][c0][:],
                                     start=(k == 0), stop=(k == KH - 1))
                    nc.tensor.matmul(psq0[:], ones128[:], sq0[:],
                                     start=(k == 0), stop=(k == KH - 1))
                    nc.tensor.matmul(pss1[:], ones128[:], x[k][c1][:],
                                     start=(k == 0), stop=(k == KH - 1))
                    nc.tensor.matmul(psq1[:], ones128[:], sq1[:],
                                     start=(k == 0), stop=(k == KH - 1))
                st0 = stats_chain(pss0, psq0, eps)
                st1 = stats_chain(pss1, psq1, eps)
                return st0, st1

            # ---------------- blocks ----------------
            for l in range(L):
                w1 = []
                w2 = []
                for k in range(KH):
                    w = wp.tile([128, H], BF16, tag=f"w1_{k}")
                    nc.gpsimd.dma_start(w[:], W1d[l, k * 128:(k + 1) * 128, :])
                    w1.append(w)
                for k in range(KH):
                    w = wp.tile([128, H], BF16, tag=f"w2_{k}")
                    nc.gpsimd.dma_start(w[:], W2d[l, k * 128:(k + 1) * 128, :])
                    w2.append(w)
                w1s_row = sp.tile([1, H], BF16, tag="w1s_row", bufs=2)
                if not fast:
                    b1row = sp.tile([1, H], BF16, tag="b1row", bufs=1)
                    nc.gpsimd.dma_start(b1row[:], b1d[l:l + 1, :])
                    b2row = sp.tile([1, H], BF16, tag="b2row", bufs=1)
                    nc.gpsimd.dma_start(b2row[:], b2d[l:l + 1, :])
                    cvec_row = sp.tile([1, H], BF16, tag="cvec_row", bufs=1)
                for half in range(2):
                    g_lhs = (ones128[:, 0:1] if fast else
                             glb_bf[:, 2 * (KH * l):2 * (KH * l) + 1])
                    psg = psB.tile([1, 512], F32, tag="small")
                    for k in range(KH):
                        nc.tensor.matmul(
                            psg[:],
                            ones128[:, 0:1] if fast else
                            glb_bf[:, 2 * (KH * l + k):2 * (KH * l + k) + 1],
                            w1[k][:, half * 512:(half + 1) * 512],
                            start=(k == 0), stop=(k == KH - 1))
                    nc.scalar.activation(w1s_row[0:1, half * 512:(half + 1) * 512],
                                         psg[:], AF.Copy)
                    if not fast:
                        psb_ = psB.tile([1, 512], F32, tag="small")
                        for k in range(KH):
                            nc.tensor.matmul(
                                psb_[:],
                                glb_bf[:, 2 * (KH * l + k) + 1:2 * (KH * l + k) + 2],
                                w1[k][:, half * 512:(half + 1) * 512],
                                start=(k == 0), stop=(k == KH - 1))
                        nc.vector.tensor_tensor(
                            cvec_row[0:1, half * 512:(half + 1) * 512], psb_[:],
                            b1row[:, half * 512:(half + 1) * 512], op=ALU.add)
                if not fast:
                    # W1 <- g * W1 (in place)
                    for k in range(KH):
                        nc.vector.tensor_scalar(
                            w1[k][:], w1[k][:],
                            glb_f[:, 2 * (KH * l + k):2 * (KH * l + k) + 1],
                            None, op0=ALU.mult)

                for c0 in range(0, CH, 2):
                    c1 = c0 + 1
                    (q0, negm0, srow0), (q1, negm1, srow1) = \
                        emit_stats_pair(c0, c1, EPS_LN)
                    r0_t = []
                    r1_t = []
                    for n in range(KH):
                        psZ0 = psA.tile([128, CW], F32, tag="ps", name="psZ0")
                        psZ1 = psA.tile([128, CW], F32, tag="ps", name="psZ1")
                        for k in range(KH):
                            nc.tensor.matmul(psZ0[:],
                                             w1[k][:, n * 128:(n + 1) * 128],
                                             x[k][c0][:], start=(k == 0),
                                             stop=False)
                            nc.tensor.matmul(psZ1[:],
                                             w1[k][:, n * 128:(n + 1) * 128],
                                             x[k][c1][:], start=(k == 0),
                                             stop=False)
                        nc.tensor.matmul(psZ0[:],
                                         w1s_row[0:1, n * 128:(n + 1) * 128],
                                         negm0[:], start=False, stop=fast)
                        nc.tensor.matmul(psZ1[:],
                                         w1s_row[0:1, n * 128:(n + 1) * 128],
                                         negm1[:], start=False, stop=fast)
                        if not fast:
                            nc.tensor.matmul(
                                psZ0[:], cvec_row[0:1, n * 128:(n + 1) * 128],
                                srow0[:], start=False, stop=True)
                            nc.tensor.matmul(
                                psZ1[:], cvec_row[0:1, n * 128:(n + 1) * 128],
                                srow1[:], start=False, stop=True)
                        r0 = rp.tile([128, CW], BF16, tag=f"r0_{n}", bufs=1,
                                     name=f"r0_{n}")
                        nc.scalar.activation(r0[:], psZ0[:], AF.Relu)
                        r0_t.append(r0)
                        r1 = rp.tile([128, CW], BF16, tag=f"r1_{n}", bufs=1,
                                     name=f"r1_{n}")
                        nc.scalar.activation(r1[:], psZ1[:], AF.Relu)
                        r1_t.append(r1)
                    for n2 in range(KH):
                        psY0 = psA.tile([128, CW], F32, tag="ps", name="psY0")
                        psY1 = psA.tile([128, CW], F32, tag="ps", name="psY1")
                        for n in range(KH):
                            nc.tensor.matmul(psY0[:],
                                             w2[n][:, n2 * 128:(n2 + 1) * 128],
                                             r0_t[n][:], start=(n == 0),
                                             stop=(fast and n == KH - 1))
                            nc.tensor.matmul(psY1[:],
                                             w2[n][:, n2 * 128:(n2 + 1) * 128],
                                             r1_t[n][:], start=(n == 0),
                                             stop=(fast and n == KH - 1))
                        if not fast:
                            nc.tensor.matmul(psY0[:],
                                             b2row[:, n2 * 128:(n2 + 1) * 128],
                                             srow0[:], start=False, stop=True)
                            nc.tensor.matmul(psY1[:],
                                             b2row[:, n2 * 128:(n2 + 1) * 128],
                                             srow1[:], start=False, stop=True)
                        t0 = ap.tile([128, CW], BF16, tag="t", name="t0")
                        nc.vector.tensor_tensor(t0[:], psY0[:], q0[:],
                                                op=ALU.mult)
                        nc.vector.tensor_tensor(x[n2][c0][:], x[n2][c0][:],
                                                t0[:], op=ALU.add)
                        t1 = ap.tile([128, CW], BF16, tag="t", name="t1")
                        nc.vector.tensor_tensor(t1[:], psY1[:], q1[:],
                                                op=ALU.mult)
                        nc.vector.tensor_tensor(x[n2][c1][:], x[n2][c1][:],
                                                t1[:], op=ALU.add)

            # ---------------- heads ----------------
            wh = []
            for k in range(KH):
                w = wp.tile([128, 2 * A], BF16, tag=f"wh_{k}", bufs=1)
                nc.gpsimd.dma_start(w[:, 0:A], Wmud[k * 128:(k + 1) * 128, :])
                nc.gpsimd.dma_start(w[:, A:2 * A], Wstdd[k * 128:(k + 1) * 128, :])
                wh.append(w)
            whs_row = sp.tile([1, 2 * A], BF16, tag="whs_row", bufs=1)
            pshg = psB.tile([1, 512], F32, tag="small")
            for k in range(KH):
                nc.tensor.matmul(pshg[:, 0:2 * A],
                                 ones128[:, 0:1] if fast else
                                 pglb_bf[:, 2 * k:2 * k + 1],
                                 wh[k][:], start=(k == 0), stop=(k == KH - 1))
            nc.scalar.activation(whs_row[:], pshg[:, 0:2 * A], AF.Copy)
            if not fast:
                cvech_row = sp.tile([1, 2 * A], BF16, tag="cvech_row", bufs=1)
                pshb = psB.tile([1, 512], F32, tag="small")
                for k in range(KH):
                    nc.tensor.matmul(pshb[:, 0:2 * A],
                                     pglb_bf[:, 2 * k + 1:2 * k + 2],
                                     wh[k][:], start=(k == 0), stop=(k == KH - 1))
                nc.vector.tensor_tensor(cvech_row[:], pshb[:, 0:2 * A],
                                        bhead[:], op=ALU.add)

            for c0 in range(0, CH, 2):
                hstats = emit_stats_pair(c0, c0 + 1, EPS_LN)
                for cc in range(2):
                  c = c0 + cc
                  q_b, negm_row, s_row = hstats[cc]
                  for j in range(4):
                    # per-row 1/std as a column: transpose a q_b block
                    pst = psB.tile([128, 128], F32, tag="small", name="pst")
                    nc.tensor.transpose(pst[:], q_b[:, j * 128:(j + 1) * 128],
                                        ident[:])
                    qcol = sp.tile([128, 1], F32, tag="qcol")
                    nc.vector.tensor_copy(qcol[:], pst[:, 0:1])

                    psH = psA.tile([128, 2 * A], F32, tag="ps")
                    for k in range(KH):
                        nc.tensor.matmul(psH[:],
                                         x[k][c][:, j * 128:(j + 1) * 128],
                                         wh[k][:], start=(k == 0), stop=False)
                    nc.tensor.matmul(psH[:],
                                     negm_row[0:1, j * 128:(j + 1) * 128],
                                     whs_row[:], start=False, stop=fast)
                    if not fast:
                        nc.tensor.matmul(psH[:],
                                         s_row[0:1, j * 128:(j + 1) * 128],
                                         cvech_row[:], start=False, stop=True)
                    outt = ap.tile([128, 2 * A], F32, tag="outt")
                    nc.vector.tensor_scalar(outt[:, 0:A], psH[:, 0:A], qcol[:],
                                            -5.0, op0=ALU.mult, op1=ALU.max)
                    nc.vector.tensor_scalar_min(outt[:, 0:A], outt[:, 0:A], 5.0)
                    nc.vector.tensor_scalar(outt[:, A:2 * A], psH[:, A:2 * A],
                                            qcol[:], 1.0, op0=ALU.mult,
                                            op1=ALU.min)
                    nc.vector.tensor_scalar_max(outt[:, A:2 * A],
                                                outt[:, A:2 * A], -5.0)
                    nc.scalar.activation(outt[:, A:2 * A], outt[:, A:2 * A],
                                         AF.Exp)
                    nc.sync.dma_start(
                        outd[(c * 4 + j) * 128:(c * 4 + j + 1) * 128, :],
                        outt[:])

    nc.compile()
    return nc


def _get_compiled(fast=True):
    if fast not in _COMPILED:
        _COMPILED[fast] = _build(fast)
    return _COMPILED[fast]


def _fast_ok(inputs):
    z = lambda k: not np.any(np.asarray(inputs[k]))
    o = lambda k: np.all(np.asarray(inputs[k]) == 1.0)
    return (z("b_in") and z("ln_b") and z("b1") and z("b2") and z("post_b")
            and z("bmu") and z("bstd") and o("ln_g") and o("post_g"))


def kernel(**inputs):
    nc = _get_compiled(fast=_fast_ok(inputs))
    f = lambda k: np.ascontiguousarray(np.asarray(inputs[k], dtype=np.float32))
    shared = {
        "W_in": f("W_in"),
        "b_in": f("b_in").reshape(1, H),
        "ln_g": f("ln_g"),
        "ln_b": f("ln_b"),
        "W1": f("W1"),
        "b1": f("b1"),
        "W2": f("W2"),
        "b2": f("b2"),
        "post_g": f("post_g").reshape(1, H),
        "post_b": f("post_b").reshape(1, H),
        "Wmu": f("Wmu"),
        "bmu": f("bmu").reshape(1, A),
        "Wstd": f("Wstd"),
        "bstd": f("bstd").reshape(1, A),
    }
    state = f("state")
    in_maps = []
    for i in range(NCORES):
        m = dict(shared)
        m["state"] = state[i * R:(i + 1) * R]
        in_maps.append(m)
    res = run_bass_kernel_spmd(nc, in_maps, core_ids=list(range(NCORES)))
    global LAST_RESULT
    LAST_RESULT = res
    full = np.concatenate([res.results[i]["out"] for i in range(NCORES)], axis=0)
    return full[:, :A].copy(), full[:, A:].copy()


LAST_RESULT = None


# revision 21
# speedup vs baseline: 1.3491x; 1.0036x over previous
"""Trainium2 Bass kernel for nn_Actor (RSNorm -> Linear -> 4x residual LN-MLP
blocks -> post-LN -> clipped mu/std heads), data-parallel over batch on 8
NeuronCores.

Strategy:
- Shard batch B=16384 into 8x2048 rows; weights replicated per core.
- RSNorm (Welford scan over batch) == population mean/var over batch; computed
  via per-shard bn_stats merged across cores with a tiny (4KB) AllReduce.
- All norms are folded into the adjacent matmuls: per-feature affine goes into
  the weight matrix, per-row (mean, std) corrections enter the PSUM
  accumulation as rank-2 matmuls, and the per-row 1/std scale commutes with
  ReLU so it is applied once on the residual update.
- Activations live feature-major ([feat partitions x row free]) so the whole
  residual trunk needs zero transposes; the heads flip back to row-major by
  using the activation tiles as the stationary matmul operand.
- Matmul compute in bf16 (fp32 PSUM accumulate); residual stream stored bf16.
"""

import sys

if "/opt/trn_rl_repo" not in sys.path:
    sys.path.insert(0, "/opt/trn_rl_repo")

import numpy as np

import concourse.bass as bass
import concourse.bacc as bacc
import concourse.mybir as mybir
from concourse import tile
from concourse.bass_utils import run_bass_kernel_spmd

# bass_utils imports antenv.axon_hooks when tracing is requested via
# BASS_TRACE; provide a no-op fallback module when the image lacks it.
try:
    import antenv.axon_hooks  # noqa: F401
except Exception:
    try:
        import types as _types
        import antenv as _antenv

        _m = _types.ModuleType("antenv.axon_hooks")
        _m.get_axon_ntff_profile_hook = lambda: None
        _m.set_axon_ntff_profile_hook = lambda h: None
        _antenv.axon_hooks = _m
        sys.modules["antenv.axon_hooks"] = _m
    except Exception:
        pass

F32 = mybir.dt.float32
BF16 = mybir.dt.bfloat16
AF = mybir.ActivationFunctionType
ALU = mybir.AluOpType

B, DIN, H, A, L = 16384, 512, 1024, 128, 4
NCORES = 8
R = B // NCORES          # 2048 rows per core
CH = 4                   # chunks per core
CW = R // CH             # 512 rows per chunk
KD = DIN // 128          # 4 k-tiles of the input dim
KH = H // 128            # 8 k-tiles of the hidden dim
EPS_RS = 1e-5
EPS_LN = 1e-5

_COMPILED = {}


def _build(fast):
    """fast=True assumes ln_g/post_g == 1 and every bias == 0 (the
    distributions pinned by the problem spec); kernel() verifies before
    dispatching here."""
    nc = bacc.Bacc("TRN2", target_bir_lowering=False, debug=False,
                   num_devices=NCORES)

    stated = nc.dram_tensor("state", [R, DIN], F32, kind="ExternalInput")
    W_ind = nc.dram_tensor("W_in", [DIN, H], F32, kind="ExternalInput")
    b_ind = nc.dram_tensor("b_in", [1, H], F32, kind="ExternalInput")
    ln_gd = nc.dram_tensor("ln_g", [L, H], F32, kind="ExternalInput")
    ln_bd = nc.dram_tensor("ln_b", [L, H], F32, kind="ExternalInput")
    W1d = nc.dram_tensor("W1", [L, H, H], F32, kind="ExternalInput")
    b1d = nc.dram_tensor("b1", [L, H], F32, kind="ExternalInput")
    W2d = nc.dram_tensor("W2", [L, H, H], F32, kind="ExternalInput")
    b2d = nc.dram_tensor("b2", [L, H], F32, kind="ExternalInput")
    post_gd = nc.dram_tensor("post_g", [1, H], F32, kind="ExternalInput")
    post_bd = nc.dram_tensor("post_b", [1, H], F32, kind="ExternalInput")
    Wmud = nc.dram_tensor("Wmu", [H, A], F32, kind="ExternalInput")
    bmud = nc.dram_tensor("bmu", [1, A], F32, kind="ExternalInput")
    Wstdd = nc.dram_tensor("Wstd", [H, A], F32, kind="ExternalInput")
    bstdd = nc.dram_tensor("bstd", [1, A], F32, kind="ExternalInput")
    outd = nc.dram_tensor("out", [R, 2 * A], F32, kind="ExternalOutput")

    identd = nc.inline_tensor(np.eye(128, dtype=np.float32), name="ident")

    # register 1e-5 as a const AP so activation(bias=eps) resolves
    eps_t = nc.alloc_sbuf_tensor("const-eps", [128, 1], F32)
    nc.gpsimd.memset(eps_t.ap(), EPS_LN)
    nc.const_aps.aps[(F32, EPS_LN)] = eps_t.ap()
    nc.all_engine_barrier()

    with tile.TileContext(nc) as tc:
        with (
            tc.tile_pool(name="const", bufs=1) as cp,
            tc.tile_pool(name="xp", bufs=1) as xp,
            tc.tile_pool(name="wp", bufs=2) as wp,
            tc.tile_pool(name="ap", bufs=3) as ap,
            tc.tile_pool(name="rp", bufs=2) as rp,
            tc.tile_pool(name="sp", bufs=4) as sp,
            tc.tile_pool(name="psA", bufs=6, space="PSUM") as psA,
            tc.tile_pool(name="psB", bufs=2, space="PSUM") as psB,
            tc.tile_pool(name="dp", bufs=1, space="DRAM") as dp,
        ):
            # ---------------- constants ----------------
            ones128 = cp.tile([128, 128], BF16)
            nc.vector.memset(ones128[:], 1.0)
            onesrow = cp.tile([1, CW], BF16)
            nc.vector.memset(onesrow[:], 1.0)
            ident = cp.tile([128, 128], F32)
            nc.sync.dma_start(ident[:], identd[:])
            identb = cp.tile([128, 128], BF16)
            nc.gpsimd.dma_start(identb[:], identd[:])

            if not fast:
                # ln_g/ln_b interleaved column tiles: [128, L*KH*2]
                glb_bf = cp.tile([128, L * KH * 2], BF16)
                glb_f = cp.tile([128, L * KH * 2], F32)
                for src, off in ((ln_gd, 0), (ln_bd, 1)):
                    view = src[:].rearrange("l (k p) -> p (l k)", p=128)
                    dst_bf = glb_bf[:].rearrange("p (lk two) -> p lk two", two=2)
                    dst_f = glb_f[:].rearrange("p (lk two) -> p lk two", two=2)
                    nc.gpsimd.dma_start(dst_bf[:, :, off], view)
                    nc.gpsimd.dma_start(dst_f[:, :, off], view)
                # post_g/post_b column tiles: [128, KH*2]
                pglb_bf = cp.tile([128, KH * 2], BF16)
                for src, off in ((post_gd, 0), (post_bd, 1)):
                    view = src[:].rearrange("o (k p) -> p (o k)", p=128)
                    dst = pglb_bf[:].rearrange("p (k two) -> p k two", two=2)
                    nc.gpsimd.dma_start(dst[:, :, off], view)
                # bias rows
                b_in_row = cp.tile([1, H], F32)
                nc.sync.dma_start(b_in_row[:], b_ind[:])
                bhead = cp.tile([1, 2 * A], F32)
                nc.sync.dma_start(bhead[:, 0:A], bmud[:])
                nc.sync.dma_start(bhead[:, A:2 * A], bstdd[:])

            # ---------------- stage A: state load + transpose ----------------
            xt = [[xp.tile([128, CW], BF16, tag=f"xt_{k}_{c}", name=f"xt_{k}_{c}")
                   for c in range(CH)] for k in range(KD)]
            # state row tiles (bf16) + per-feature batch sums on the PE
            ps_rsum = psA.tile([128, DIN], F32, tag="ps", name="ps_rsum")
            ps_rsq = psA.tile([128, DIN], F32, tag="ps", name="ps_rsq")
            srows = []
            for c in range(CH):
                for j in range(4):
                    idx = c * 4 + j
                    srow = ap.tile([128, DIN], BF16, tag=f"srow{idx % 4}",
                                   bufs=1, name=f"srow_{c}_{j}")
                    nc.gpsimd.dma_start(
                        srow[:], stated[idx * 128:(idx + 1) * 128, :])
                    sqr = ap.tile([128, DIN], BF16, tag="sqr", name="sqr")
                    nc.scalar.activation(sqr[:], srow[:], AF.Square)
                    nc.tensor.matmul(ps_rsum[:], ones128[:], srow[:],
                                     start=(idx == 0), stop=(idx == CH * 4 - 1))
                    nc.tensor.matmul(ps_rsq[:], ones128[:], sqr[:],
                                     start=(idx == 0), stop=(idx == CH * 4 - 1))
                    srows.append((c, j, srow))
            # transpose state tiles on the PE (bf16, 1 cyc/row)
            for c, j, srow in srows:
                for k in range(KD):
                    pst = psA.tile([128, 128], BF16, tag="ps", name="pstr")
                    nc.tensor.transpose(pst[:], srow[:, k * 128:(k + 1) * 128],
                                        identb[:])
                    nc.vector.tensor_copy(xt[k][c][:, j * 128:(j + 1) * 128],
                                          pst[:])

            # ---------------- rsnorm allreduce ----------------
            rsum_row = sp.tile([1, DIN], F32, tag="rsum_row", bufs=1)
            nc.scalar.activation(rsum_row[:], ps_rsum[0:1, :], AF.Copy)
            rsq_row = sp.tile([1, DIN], F32, tag="rsq_row", bufs=1)
            nc.scalar.activation(rsq_row[:], ps_rsq[0:1, :], AF.Copy)
            cc_in = dp.tile([2, DIN], F32)
            cc_out = dp.tile([2, DIN], F32, addr_space="Shared")
            nc.sync.dma_start(cc_in[0:1, :], rsum_row[:])
            nc.sync.dma_start(cc_in[1:2, :], rsq_row[:])
            nc.gpsimd.collective_compute(
                "AllReduce", ALU.add,
                replica_groups=[list(range(NCORES))],
                ins=[cc_in[:].opt()], outs=[cc_out[:].opt()])
            allout = sp.tile([2, DIN], F32, tag="allout", bufs=1)
            nc.gpsimd.dma_start(allout[:], cc_out[:])

            # per-feature fold factors for W_in: transpose [2,128] stat
            # blocks into [128,2] columns, then the scalar chain
            a_col = []
            c_col = []
            for k in range(KD):
                pstc = psB.tile([128, 2], F32, tag="small", name="pstc")
                nc.tensor.transpose(pstc[:],
                                    allout[0:2, k * 128:(k + 1) * 128],
                                    ident[0:2, 0:2])
                stc = sp.tile([128, 2], F32, tag=f"stc_{k}", bufs=1)
                nc.vector.tensor_copy(stc[:], pstc[:])
                muk = sp.tile([128, 1], F32, tag=f"muk_{k}", bufs=1)
                nc.scalar.activation(muk[:], stc[:, 0:1],
                                     AF.Copy, scale=1.0 / B)
                var = sp.tile([128, 1], F32, tag="var1")
                nc.scalar.activation(var[:], stc[:, 1:2],
                                     AF.Copy, scale=1.0 / B)
                msq = sp.tile([128, 1], F32, tag="msq1")
                nc.vector.tensor_tensor(msq[:], muk[:], muk[:], op=ALU.mult)
                nc.vector.tensor_tensor(var[:], var[:], msq[:], op=ALU.subtract)
                nc.vector.tensor_scalar_max(var[:], var[:], 0.001)
                ak = sp.tile([128, 1], F32, tag=f"ak_{k}", bufs=1)
                nc.scalar.activation(ak[:], var[:], AF.Abs_reciprocal_sqrt,
                                     bias=EPS_RS)
                mak = sp.tile([128, 1], F32, tag="mak")
                nc.vector.tensor_tensor(mak[:], muk[:], ak[:], op=ALU.mult)
                ck = sp.tile([128, 1], BF16, tag=f"ck_{k}", bufs=1)
                nc.scalar.activation(ck[:], mak[:], AF.Copy, scale=-1.0)
                a_col.append(ak)
                c_col.append(ck)

            # ---------------- W_in fold + x1 ----------------
            w_in = []
            for k in range(KD):
                w = wp.tile([128, H], BF16, tag=f"win_{k}", bufs=1)
                nc.gpsimd.dma_start(w[:], W_ind[k * 128:(k + 1) * 128, :])
                w_in.append(w)
            # dvec = c @ W_in + b_in
            dvec = sp.tile([1, H], BF16, tag="dvec", bufs=1)
            for half in range(2):
                psd = psB.tile([2, 512], F32, tag="small")
                for k in range(KD):
                    nc.tensor.matmul(psd[0:1, :], c_col[k][:],
                                     w_in[k][:, half * 512:(half + 1) * 512],
                                     start=(k == 0), stop=(k == KD - 1))
                if fast:
                    nc.scalar.activation(dvec[:, half * 512:(half + 1) * 512],
                                         psd[0:1, :], AF.Copy)
                else:
                    nc.vector.tensor_tensor(
                        dvec[:, half * 512:(half + 1) * 512], psd[0:1, :],
                        b_in_row[:, half * 512:(half + 1) * 512], op=ALU.add)
            # W_in <- a * W_in (in place, after dvec matmuls)
            for k in range(KD):
                nc.vector.tensor_scalar(w_in[k][:], w_in[k][:], a_col[k][:],
                                        None, op0=ALU.mult)

            x = [[xp.tile([128, CW], BF16, tag=f"x_{n}_{c}", name=f"x_{n}_{c}")
                  for c in range(CH)] for n in range(KH)]

            # ---------------- helper: per-chunk-pair row stats ----------------
            def stats_chain(pss, psq, eps):
                negm_row = sp.tile([1, CW], BF16, tag="negm", bufs=3,
                                   name="negm_row")
                nc.scalar.activation(negm_row[:], pss[0:1, :], AF.Copy,
                                     scale=-1.0 / H)
                m_b = ap.tile([128, CW], F32, tag="m_b", bufs=2, name="m_b")
                nc.scalar.activation(m_b[:], pss[:], AF.Copy, scale=1.0 / H)
                nc.vector.tensor_tensor(m_b[:], m_b[:], m_b[:], op=ALU.mult)
                var = ap.tile([128, CW], F32, tag="varb", bufs=2, name="var")
                nc.vector.scalar_tensor_tensor(var[:], psq[:], 1.0 / H, m_b[:],
                                               op0=ALU.mult, op1=ALU.subtract)
                q_b = ap.tile([128, CW], F32, tag="qb", bufs=3, name="q_b")
                nc.scalar.activation(q_b[:], var[:], AF.Abs_reciprocal_sqrt,
                                     bias=eps)
                if fast:
                    return q_b, negm_row, None
                vpe = ap.tile([1, CW], F32, tag="vpe", bufs=1, name="vpe")
                nc.vector.tensor_scalar_add(vpe[:], var[0:1, :], eps)
                s_row = sp.tile([1, CW], BF16, tag="s_row", bufs=2,
                                name="s_row")
                nc.vector.tensor_tensor(s_row[:], vpe[:], q_b[0:1, :],
                                        op=ALU.mult)
                return q_b, negm_row, s_row

            def emit_stats_pair(c0, c1, eps):
                pss0 = psA.tile([128, CW], F32, tag="ps", name="pss0")
                psq0 = psA.tile([128, CW], F32, tag="ps", name="psq0")
                pss1 = psA.tile([128, CW], F32, tag="ps", name="pss1")
                psq1 = psA.tile([128, CW], F32, tag="ps", name="psq1")
                for k in range(KH):
                    sq0 = ap.tile([128, CW], BF16, tag="sq", name="sq0")
                    nc.scalar.activation(sq0[:], x[k][c0][:], AF.Square)
                    sq1 = ap.tile([128, CW], BF16, tag="sq", name="sq1")
                    nc.scalar.activation(sq1[:], x[k][c1][:], AF.Square)
                    nc.tensor.matmul(pss0[:], ones128[:], x[k][c0][:],
                                     start=(k == 0), stop=(k == KH - 1))
                    nc.tensor.matmul(psq0[:], ones128[:], sq0[:],
                                     start=(k == 0), stop=(k == KH - 1))
                    nc.tensor.matmul(pss1[:], ones128[:], x[k][c1][:],
                                     start=(k == 0), stop=(k == KH - 1))
                    nc.tensor.matmul(psq1[:], ones128[:], sq1[:],
                                     start=(k == 0), stop=(k == KH - 1))
                st0 = stats_chain(pss0, psq0, eps)
                st1 = stats_chain(pss1, psq1, eps)
                return st0, st1

            # ---------------- block prep (weights + folded rows) ----------------
            def prep_block(l):
                w1 = []
                w2 = []
                for k in range(KH):
                    w = wp.tile([128, H], BF16, tag=f"w1_{k}", name=f"w1_{l}_{k}")
                    nc.gpsimd.dma_start(w[:], W1d[l, k * 128:(k + 1) * 128, :])
                    w1.append(w)
                for k in range(KH):
                    w = wp.tile([128, H], BF16, tag=f"w2_{k}", name=f"w2_{l}_{k}")
                    nc.gpsimd.dma_start(w[:], W2d[l, k * 128:(k + 1) * 128, :])
                    w2.append(w)
                w1s_row = sp.tile([1, H], BF16, tag="w1s_row", bufs=2,
                                  name=f"w1s_{l}")
                b2row = cvec_row = None
                if not fast:
                    b1row = sp.tile([1, H], BF16, tag="b1row", bufs=1,
                                    name=f"b1r_{l}")
                    nc.gpsimd.dma_start(b1row[:], b1d[l:l + 1, :])
                    b2row = sp.tile([1, H], BF16, tag="b2row", bufs=1,
                                    name=f"b2r_{l}")
                    nc.gpsimd.dma_start(b2row[:], b2d[l:l + 1, :])
                    cvec_row = sp.tile([1, H], BF16, tag="cvec_row", bufs=1,
                                       name=f"cvec_{l}")
                for half in range(2):
                    psg = psB.tile([1, 512], F32, tag="small", name="psg")
                    for k in range(KH):
                        nc.tensor.matmul(
                            psg[:],
                            ones128[:, 0:1] if fast else
                            glb_bf[:, 2 * (KH * l + k):2 * (KH * l + k) + 1],
                            w1[k][:, half * 512:(half + 1) * 512],
                            start=(k == 0), stop=(k == KH - 1))
                    nc.scalar.activation(w1s_row[0:1, half * 512:(half + 1) * 512],
                                         psg[:], AF.Copy)
                    if not fast:
                        psb_ = psB.tile([1, 512], F32, tag="small", name="psb_")
                        for k in range(KH):
                            nc.tensor.matmul(
                                psb_[:],
                                glb_bf[:, 2 * (KH * l + k) + 1:2 * (KH * l + k) + 2],
                                w1[k][:, half * 512:(half + 1) * 512],
                                start=(k == 0), stop=(k == KH - 1))
                        nc.vector.tensor_tensor(
                            cvec_row[0:1, half * 512:(half + 1) * 512], psb_[:],
                            b1row[:, half * 512:(half + 1) * 512], op=ALU.add)
                if not fast:
                    # W1 <- g * W1 (in place)
                    for k in range(KH):
                        nc.vector.tensor_scalar(
                            w1[k][:], w1[k][:],
                            glb_f[:, 2 * (KH * l + k):2 * (KH * l + k) + 1],
                            None, op0=ALU.mult)
                return w1, w2, w1s_row, cvec_row, b2row

            # hoist blocks 0/1 prep into the allreduce-wait window
            preps = {0: prep_block(0), 1: prep_block(1)}

            # ---------------- x1 = folded-rsnorm state @ W_in ----------------
            for c0 in range(0, CH, 2):
                c1 = c0 + 1
                for n in range(KH):
                    ps0 = psA.tile([128, CW], F32, tag="ps", name="ps0")
                    ps1 = psA.tile([128, CW], F32, tag="ps", name="ps1")
                    for k in range(KD):
                        nc.tensor.matmul(ps0[:], w_in[k][:, n * 128:(n + 1) * 128],
                                         xt[k][c0][:], start=(k == 0), stop=False)
                        nc.tensor.matmul(ps1[:], w_in[k][:, n * 128:(n + 1) * 128],
                                         xt[k][c1][:], start=(k == 0), stop=False)
                    nc.tensor.matmul(ps0[:], dvec[:, n * 128:(n + 1) * 128],
                                     onesrow[:], start=False, stop=True)
                    nc.tensor.matmul(ps1[:], dvec[:, n * 128:(n + 1) * 128],
                                     onesrow[:], start=False, stop=True)
                    nc.scalar.activation(x[n][c0][:], ps0[:], AF.Copy)
                    nc.scalar.activation(x[n][c1][:], ps1[:], AF.Copy)

            # ---------------- blocks ----------------
            for l in range(L):
                w1, w2, w1s_row, cvec_row, b2row = (
                    preps[l] if l in preps else prep_block(l))

                for c0 in range(0, CH, 2):
                    c1 = c0 + 1
                    (q0, negm0, srow0), (q1, negm1, srow1) = \
                        emit_stats_pair(c0, c1, EPS_LN)
                    r0_t = []
                    r1_t = []
                    for n in range(KH):
                        psZ0 = psA.tile([128, CW], F32, tag="ps", name="psZ0")
                        psZ1 = psA.tile([128, CW], F32, tag="ps", name="psZ1")
                        for k in range(KH):
                            nc.tensor.matmul(psZ0[:],
                                             w1[k][:, n * 128:(n + 1) * 128],
                                             x[k][c0][:], start=(k == 0),
                                             stop=False)
                            nc.tensor.matmul(psZ1[:],
                                             w1[k][:, n * 128:(n + 1) * 128],
                                             x[k][c1][:], start=(k == 0),
                                             stop=False)
                        nc.tensor.matmul(psZ0[:],
                                         w1s_row[0:1, n * 128:(n + 1) * 128],
                                         negm0[:], start=False, stop=fast)
                        nc.tensor.matmul(psZ1[:],
                                         w1s_row[0:1, n * 128:(n + 1) * 128],
                                         negm1[:], start=False, stop=fast)
                        if not fast:
                            nc.tensor.matmul(
                                psZ0[:], cvec_row[0:1, n * 128:(n + 1) * 128],
                                srow0[:], start=False, stop=True)
                            nc.tensor.matmul(
                                psZ1[:], cvec_row[0:1, n * 128:(n + 1) * 128],
                                srow1[:], start=False, stop=True)
                        r0 = rp.tile([128, CW], BF16, tag=f"r0_{n}", bufs=1,
                                     name=f"r0_{n}")
                        nc.scalar.activation(r0[:], psZ0[:], AF.Relu)
                        r0_t.append(r0)
                        r1 = rp.tile([128, CW], BF16, tag=f"r1_{n}", bufs=1,
                                     name=f"r1_{n}")
                        nc.scalar.activation(r1[:], psZ1[:], AF.Relu)
                        r1_t.append(r1)
                    for n2 in range(KH):
                        psY0 = psA.tile([128, CW], F32, tag="ps", name="psY0")
                        psY1 = psA.tile([128, CW], F32, tag="ps", name="psY1")
                        for n in range(KH):
                            nc.tensor.matmul(psY0[:],
                                             w2[n][:, n2 * 128:(n2 + 1) * 128],
                                             r0_t[n][:], start=(n == 0),
                                             stop=(fast and n == KH - 1))
                            nc.tensor.matmul(psY1[:],
                                             w2[n][:, n2 * 128:(n2 + 1) * 128],
                                             r1_t[n][:], start=(n == 0),
                                             stop=(fast and n == KH - 1))
                        if not fast:
                            nc.tensor.matmul(psY0[:],
                                             b2row[:, n2 * 128:(n2 + 1) * 128],
                                             srow0[:], start=False, stop=True)
                            nc.tensor.matmul(psY1[:],
                                             b2row[:, n2 * 128:(n2 + 1) * 128],
                                             srow1[:], start=False, stop=True)
                        t0 = ap.tile([128, CW], BF16, tag="t", name="t0")
                        nc.vector.tensor_tensor(t0[:], psY0[:], q0[:],
                                                op=ALU.mult)
                        nc.vector.tensor_tensor(x[n2][c0][:], x[n2][c0][:],
                                                t0[:], op=ALU.add)
                        t1 = ap.tile([128, CW], BF16, tag="t", name="t1")
                        nc.vector.tensor_tensor(t1[:], psY1[:], q1[:],
                                                op=ALU.mult)
                        nc.vector.tensor_tensor(x[n2][c1][:], x[n2][c1][:],
                                                t1[:], op=ALU.add)

            # ---------------- heads ----------------
            wh = []
            for k in range(KH):
                w = wp.tile([128, 2 * A], BF16, tag=f"wh_{k}", bufs=1)
                nc.gpsimd.dma_start(w[:, 0:A], Wmud[k * 128:(k + 1) * 128, :])
                nc.gpsimd.dma_start(w[:, A:2 * A], Wstdd[k * 128:(k + 1) * 128, :])
                wh.append(w)
            whs_row = sp.tile([1, 2 * A], BF16, tag="whs_row", bufs=1)
            pshg = psB.tile([1, 512], F32, tag="small")
            for k in range(KH):
                nc.tensor.matmul(pshg[:, 0:2 * A],
                                 ones128[:, 0:1] if fast else
                                 pglb_bf[:, 2 * k:2 * k + 1],
                                 wh[k][:], start=(k == 0), stop=(k == KH - 1))
            nc.scalar.activation(whs_row[:], pshg[:, 0:2 * A], AF.Copy)
            if not fast:
                cvech_row = sp.tile([1, 2 * A], BF16, tag="cvech_row", bufs=1)
                pshb = psB.tile([1, 512], F32, tag="small")
                for k in range(KH):
                    nc.tensor.matmul(pshb[:, 0:2 * A],
                                     pglb_bf[:, 2 * k + 1:2 * k + 2],
                                     wh[k][:], start=(k == 0), stop=(k == KH - 1))
                nc.vector.tensor_tensor(cvech_row[:], pshb[:, 0:2 * A],
                                        bhead[:], op=ALU.add)

            for c0 in range(0, CH, 2):
                hstats = emit_stats_pair(c0, c0 + 1, EPS_LN)
                for cc in range(2):
                  c = c0 + cc
                  q_b, negm_row, s_row = hstats[cc]
                  for j in range(4):
                    # per-row 1/std as a column: transpose a q_b block
                    pst = psB.tile([128, 128], F32, tag="small", name="pst")
                    nc.tensor.transpose(pst[:], q_b[:, j * 128:(j + 1) * 128],
                                        ident[:])
                    qcol = sp.tile([128, 1], F32, tag="qcol")
                    nc.vector.tensor_copy(qcol[:], pst[:, 0:1])

                    psH = psA.tile([128, 2 * A], F32, tag="ps")
                    for k in range(KH):
                        nc.tensor.matmul(psH[:],
                                         x[k][c][:, j * 128:(j + 1) * 128],
                                         wh[k][:], start=(k == 0), stop=False)
                    nc.tensor.matmul(psH[:],
                                     negm_row[0:1, j * 128:(j + 1) * 128],
                                     whs_row[:], start=False, stop=fast)
                    if not fast:
                        nc.tensor.matmul(psH[:],
                                         s_row[0:1, j * 128:(j + 1) * 128],
                                         cvech_row[:], start=False, stop=True)
                    outt = ap.tile([128, 2 * A], F32, tag="outt")
                    nc.vector.tensor_scalar(outt[:, 0:A], psH[:, 0:A], qcol[:],
                                            -5.0, op0=ALU.mult, op1=ALU.max)
                    nc.vector.tensor_scalar_min(outt[:, 0:A], outt[:, 0:A], 5.0)
                    nc.vector.tensor_scalar(outt[:, A:2 * A], psH[:, A:2 * A],
                                            qcol[:], 1.0, op0=ALU.mult,
                                            op1=ALU.min)
                    nc.vector.tensor_scalar_max(outt[:, A:2 * A],
                                                outt[:, A:2 * A], -5.0)
                    nc.scalar.activation(outt[:, A:2 * A], outt[:, A:2 * A],
                                         AF.Exp)
                    nc.sync.dma_start(
                        outd[(c * 4 + j) * 128:(c * 4 + j + 1) * 128, :],
                        outt[:])

    nc.compile()
    return nc


def _get_compiled(fast=True):
    if fast not in _COMPILED:
        _COMPILED[fast] = _build(fast)
    return _COMPILED[fast]


def _fast_ok(inputs):
    z = lambda k: not np.any(np.asarray(inputs[k]))
    o = lambda k: np.all(np.asarray(inputs[k]) == 1.0)
    return (z("b_in") and z("ln_b") and z("b1") and z("b2") and z("post_b")
            and z("bmu") and z("bstd") and o("ln_g") and o("post_g"))


def kernel(**inputs):
    nc = _get_compiled(fast=_fast_ok(inputs))
    f = lambda k: np.ascontiguousarray(np.asarray(inputs[k], dtype=np.float32))
    shared = {
        "W_in": f("W_in"),
        "b_in": f("b_in").reshape(1, H),
        "ln_g": f("ln_g"),
        "ln_b": f("ln_b"),
        "W1": f("W1"),
        "b1": f("b1"),
        "W2": f("W2"),
        "b2": f("b2"),
        "post_g": f("post_g").reshape(1, H),
        "post_b": f("post_b").reshape(1, H),
        "Wmu": f("Wmu"),
        "bmu": f("bmu").reshape(1, A),
        "Wstd": f("Wstd"),
        "bstd": f("bstd").reshape(1, A),
    }
    state = f("state")
    in_maps = []
    for i in range(NCORES):
        m = dict(shared)
        m["state"] = state[i * R:(i + 1) * R]
        in_maps.append(m)
    res = run_bass_kernel_spmd(nc, in_maps, core_ids=list(range(NCORES)))
    global LAST_RESULT
    LAST_RESULT = res
    full = np.concatenate([res.results[i]["out"] for i in range(NCORES)], axis=0)
    return full[:, :A].copy(), full[:, A:].copy()


LAST_RESULT = None
